# revision 15
# baseline (speedup 1.0000x reference)
"""Trainium2 Bass kernel for MinimalCopresheafTNN (GNN message passing).

Strategy (8 NeuronCores, SPMD single program):
  * Host: fold W_r / R[p] / W1 into one per-polarity matrix D_p = W_r.T @ R_p @ W1.T
    (linearity of segment_sum), fold res_scale into W2. Since b1 == 0, the
    per-node deg_norm scale cancels through LayerNorm1 (LN is invariant to
    positive per-row scaling), so it is dropped entirely. Permute nodes so
    each core owns a contiguous, polarity-grouped slice (segments padded to
    128 and uniform across cores).
  * Device, per core:
      Phase A: x_send = x @ S[pol] for the core's slice (bf16 matmuls),
               AllGather the full x_send table into HBM on every core.
      Phase B: SpMM agg = scatter-add of x_send[row] into the core's dest
               nodes: dma_gather of source rows (4 table quadrants, int16
               indices, max 1024/call) + one-hot matmul scatter into PSUM
               per 128-dest window.
      Phase C: z1 = aggT @ D_p -> LayerNorm(+ReLU on scalar engine) ->
               transpose -> @ (res*W2.T) + x -> LayerNorm -> out.
  * Host: inverse-permute per-core outputs into the full [N, D] result.
"""

import sys

import numpy as np

sys.path.insert(0, "/opt/trn_rl_repo")

NCORES = 8
LN_EPS = 1e-5
DMA_SCRATCH = 65536
GW = 4                           # windows per group (PSUM bank = 512 f32)
SUBB = 8                         # chunks per gather batch (1024 idx HW limit)
GBUFS = 12                       # G-tile pipeline depth


# ----------------------------------------------------------------------------
# host-side preparation
# ----------------------------------------------------------------------------

def _prepare(inputs):
    x = np.asarray(inputs["x"], np.float32)
    N, D = x.shape
    S = (np.asarray(inputs["send_maps"], np.float32)
         + np.asarray(inputs["delta_send"], np.float32))
    Rm = (np.asarray(inputs["receive_maps"], np.float32)
          + np.asarray(inputs["delta_receive"], np.float32))
    P = S.shape[0]
    W_r = np.asarray(inputs["W_r"], np.float32)
    W1 = np.asarray(inputs["W1"], np.float32)
    b1 = np.asarray(inputs["b1"], np.float32)
    ln1_g = np.asarray(inputs["ln1_g"], np.float32)
    ln1_b = np.asarray(inputs["ln1_b"], np.float32)
    W2 = np.asarray(inputs["W2"], np.float32)
    b2 = np.asarray(inputs["b2"], np.float32)
    norm_g = np.asarray(inputs["norm_g"], np.float32)
    norm_b = np.asarray(inputs["norm_b"], np.float32)
    res = float(np.asarray(inputs["res_scale"]))
    row = np.asarray(inputs["row"]).astype(np.int64)
    col = np.asarray(inputs["col"]).astype(np.int64)
    pols = np.asarray(inputs["ring_polarities"]).astype(np.int64) % P
    E = row.shape[0]

    deg = np.bincount(row, minlength=N).astype(np.float32)
    dn = (1.0 / np.maximum(deg, 1.0)).astype(np.float32)
    indeg = np.bincount(col, minlength=N)

    # deg_norm is a positive per-row scale applied before `@ W1 + b1` and
    # LayerNorm; when b1 == 0 LayerNorm's scale invariance cancels it.
    use_dn = not bool(np.all(b1 == 0))
    trivial_aff1 = bool(np.all(ln1_g == 1) and np.all(ln1_b == 0))
    trivial_aff2 = bool(np.all(norm_g == 1) and np.all(norm_b == 0))
    trivial_b2 = bool(np.all(b2 == 0))

    # --- node -> (core, position) assignment --------------------------------
    L = np.zeros(P, np.int64)              # padded segment length per polarity
    core_nodes = [[None] * P for _ in range(NCORES)]
    for p in range(P):
        nodes_p = np.where(pols == p)[0]
        order = nodes_p[np.argsort(-indeg[nodes_p], kind="stable")]
        mx = 0
        for c in range(NCORES):
            core_nodes[c][p] = order[c::NCORES]
            mx = max(mx, len(core_nodes[c][p]))
        L[p] = max(128, ((mx + 127) // 128) * 128)
    M = int(L.sum())
    M = ((M + 511) // 512) * 512          # quadrants must be block-aligned
    W = M // 128
    NP = NCORES * M
    MQ = M // 4
    Q = NCORES * MQ                       # rows per quadrant table
    assert Q <= 32767, f"quadrant rows {Q} exceed int16 range"

    seg_start = np.concatenate([[0], np.cumsum(L)[:-1]])
    pol_of_block = np.repeat(np.arange(P), L // 128)
    pol_of_block = np.concatenate(
        [pol_of_block, np.full(W - len(pol_of_block), P - 1, np.int64)])

    perm = np.full(NP, -1, dtype=np.int64)
    for c in range(NCORES):
        for p in range(P):
            nodes = core_nodes[c][p]
            n_w = L[p] // 128
            base = c * M + seg_start[p]
            j = np.arange(len(nodes))
            perm[base + (j % n_w) * 128 + j // n_w] = nodes
    real = perm >= 0
    pos_of = np.empty(N, dtype=np.int64)
    pos_of[perm[real]] = np.nonzero(real)[0]

    # --- edge layout --------------------------------------------------------
    col_pos = pos_of[col]
    row_pos = pos_of[row]
    core_e = col_pos // M
    w_e = (col_pos % M) // 128
    rel_e = (col_pos % M) % 128
    n_in_core = row_pos % M
    q_e = n_in_core // MQ
    rel_s = (row_pos // M) * MQ + (n_in_core % MQ)
    dn_e = dn[col]

    key = (core_e * W + w_e) * 4 + q_e
    cnt = np.bincount(key, minlength=NCORES * W * 4).reshape(NCORES, W, 4)
    C = np.maximum(1, -(-cnt.max(axis=0) // 128)).astype(np.int64)      # [W, 4]

    wgroups = [list(range(g, min(g + GW, W))) for g in range(0, W, GW)]

    chunk_start = np.zeros((W, 4), np.int64)
    chunk_w, chunk_q, chunk_k = [], [], []
    batches_by_group = []            # [gi] -> list of (q, ch0, ch1), <= SUBB
    group_ch0 = []                   # first chunk id of each group
    nch = 0
    for wg in wgroups:
        group_ch0.append(nch)
        gb = []
        for q in range(4):
            b0 = nch
            for w in wg:
                chunk_start[w, q] = nch
                for k in range(C[w, q]):
                    chunk_w.append(w)
                    chunk_q.append(q)
                    chunk_k.append(k)
                nch += C[w, q]
            for s0 in range(b0, nch, SUBB):
                gb.append((q, s0, min(s0 + SUBB, nch)))
        batches_by_group.append(gb)
    NCH = int(nch)
    EP = 128 * NCH
    group_nch = [(batches_by_group[gi][-1][2] - group_ch0[gi])
                 for gi in range(len(wgroups))]
    NCHG = max(group_nch)

    import ml_dtypes
    bf16 = ml_dtypes.bfloat16
    f8 = ml_dtypes.float8_e4m3
    idx_arr = np.zeros((NCORES, EP), np.int16)
    reld_arr = np.full((NCORES, 128, NCH), -1, np.int16)

    order_e = np.argsort(key, kind="stable")
    counts_flat = np.bincount(key, minlength=NCORES * W * 4)
    group_start = np.zeros(NCORES * W * 4 + 1, np.int64)
    group_start[1:] = np.cumsum(counts_flat)
    r = np.arange(E) - group_start[key[order_e]]
    c_of = core_e[order_e]
    tchunk = chunk_start[w_e[order_e], q_e[order_e]] + r // 128
    lane = r % 128
    s = tchunk * 128 + lane
    idx_arr[c_of, s] = rel_s[order_e].astype(np.int16)
    reld_arr[c_of, lane, tchunk] = rel_e[order_e].astype(np.int16)

    # Uniform trailing trim: the Q7 gather ucode skips trailing negative
    # indices, and num_idxs_reg must equal the non-negative count — trim
    # every batch at the max-over-cores last-real-edge position (identical on
    # all cores). First GBUFS batches untouched (first-use G slots may be NaN).
    occupied = np.zeros((NCORES, EP), bool)
    occupied[c_of, s] = True
    batch_cnt_by_group = []
    for gi, gb in enumerate(batches_by_group):
        cnts = []
        for (_, ch0, ch1) in gb:
            Lb = (ch1 - ch0) * 128
            # first-emitted groups left untrimmed (first-use G slots are NaN)
            if gi < 3:
                cnts.append(Lb)
                continue
            nz = np.nonzero(occupied[:, ch0 * 128:ch1 * 128].any(axis=0))[0]
            T = int(nz[-1] + 1) if len(nz) else 16
            T = min(Lb, ((T + 15) // 16) * 16)
            idx_arr[:, ch0 * 128 + T:ch1 * 128] = -1
            cnts.append(T)
        batch_cnt_by_group.append(cnts)

    # host-built one-hot scatter matrix (0/1 exact in fp8): [128, NCH, 128]
    hoh = (reld_arr[:, :, :, None]
           == np.arange(128, dtype=np.int16)[None, None, None, :]).astype(f8)

    # wrapped + replicated gather-index layout: idx i lives at [i%16, i//16],
    # replicated over the 8 Q7 partition groups
    idx_rep = np.empty((NCORES, 128, EP // 16), np.int16)
    for c in range(NCORES):
        idx_rep[c] = np.tile(idx_arr[c].reshape(EP // 16, 16).T, (8, 1))

    # --- per-core node data -------------------------------------------------
    x_nm = np.zeros((NCORES, M, D), np.float32)
    pc = perm.reshape(NCORES, M)
    for c in range(NCORES):
        m = pc[c] >= 0
        x_nm[c][m] = x[pc[c][m]]
    xT = np.ascontiguousarray(x_nm.transpose(0, 2, 1)).astype(bf16)

    # per-window deg_norm column (only used when b1 != 0)
    dn_nm = np.ones((NCORES, M), np.float32)
    for c in range(NCORES):
        m = pc[c] >= 0
        dn_nm[c][m] = dn[pc[c][m]]
    dn_cols = dn_nm.reshape(NCORES, W, 128).transpose(0, 2, 1).copy()

    # --- fused weights ------------------------------------------------------
    D_all = np.einsum(
        "de,pef,fg->pdg",
        W_r.T.astype(np.float64), Rm.astype(np.float64), W1.T.astype(np.float64),
    ).astype(np.float32)
    W2s = (res * W2.T).astype(np.float32)

    cfg = dict(
        D=D, P=P, M=M, W=W, NP=NP, Q=Q, MQ=MQ, NCH=NCH, EP=EP, NCHG=NCHG,
        pol_of_block=pol_of_block.tolist(),
        wgroups=wgroups, C=C, batches_by_group=batches_by_group,
        batch_cnt_by_group=batch_cnt_by_group, group_ch0=group_ch0,
        group_nch=group_nch,
        chunk_w=chunk_w, chunk_k=chunk_k,
        use_dn=use_dn, trivial_aff1=trivial_aff1, trivial_aff2=trivial_aff2,
        trivial_b2=trivial_b2,
    )
    weights = dict(
        S_all=np.ascontiguousarray(S.reshape(P * D, D)).astype(bf16),
        D_all=np.ascontiguousarray(D_all.reshape(P * D, D)).astype(bf16),
        W2s=np.ascontiguousarray(W2s).astype(bf16),
        IDENT=np.eye(128, dtype=np.float32).astype(bf16),
        B1ROW=np.tile(b1, (128, 1)).astype(np.float32),
        G1ROW=np.tile(ln1_g, (128, 1)).astype(np.float32),
        B1LROW=np.tile(ln1_b, (128, 1)).astype(np.float32),
        GNROW=np.tile(norm_g, (128, 1)).astype(np.float32),
        BNROW=np.tile(norm_b, (128, 1)).astype(np.float32),
        B2ROW=np.tile(res * b2, (128, 1)).astype(np.float32),
    )
    in_maps = [
        dict(x_nm=x_nm[c], xT=xT[c], idx=idx_rep[c], hoh=hoh[c],
             dncol=dn_cols[c])
        for c in range(NCORES)
    ]
    return cfg, weights, in_maps, perm, N


# ----------------------------------------------------------------------------
# device program
# ----------------------------------------------------------------------------

def _build_nc(cfg, weights):
    import concourse.bass as bass
    import concourse.mybir as mybir
    import concourse.tile as tile
    from concourse import bacc

    f32 = mybir.dt.float32
    bf = mybir.dt.bfloat16
    f8 = mybir.dt.float8e4
    i16 = mybir.dt.int16
    AF = mybir.ActivationFunctionType
    D, P, M, W = cfg["D"], cfg["P"], cfg["M"], cfg["W"]
    NP, Q, NCH, EP = cfg["NP"], cfg["Q"], cfg["NCH"], cfg["EP"]
    MQ, NCHG = cfg["MQ"], cfg["NCHG"]
    pol_of_block = cfg["pol_of_block"]
    wgroups, C = cfg["wgroups"], cfg["C"]
    batches_by_group = cfg["batches_by_group"]
    batch_cnt_by_group = cfg["batch_cnt_by_group"]
    group_ch0, group_nch = cfg["group_ch0"], cfg["group_nch"]
    chunk_w, chunk_k = cfg["chunk_w"], cfg["chunk_k"]
    use_dn = cfg["use_dn"]
    aff1, aff2 = not cfg["trivial_aff1"], not cfg["trivial_aff2"]
    use_b2 = not cfg["trivial_b2"]
    NG = len(wgroups)

    nc = bacc.Bacc("TRN2", target_bir_lowering=False, debug=False,
                   num_devices=NCORES, enable_asserts=False,
                   dynamic_dma_scratch_size=DMA_SCRATCH,
                   num_swdge_queues=4)

    x_nm_t = nc.dram_tensor("x_nm", [M, D], f32, kind="ExternalInput")
    xT_t = nc.dram_tensor("xT", [D, M], bf, kind="ExternalInput")
    idx_t = nc.dram_tensor("idx", [128, EP // 16], i16, kind="ExternalInput")
    hoh_t = nc.dram_tensor("hoh", [128, NCH, 128], f8, kind="ExternalInput")
    dn_t = nc.dram_tensor("dncol", [128, W], f32, kind="ExternalInput")
    out_t = nc.dram_tensor("out", [M, D], f32, kind="ExternalOutput")

    S_c = nc.inline_tensor(weights["S_all"], name="S_all")
    D_c = nc.inline_tensor(weights["D_all"], name="D_all")
    W2_c = nc.inline_tensor(weights["W2s"], name="W2s")
    ID_c = nc.inline_tensor(weights["IDENT"], name="IDENT")
    aff_c = {}
    if aff1:
        aff_c["G1"] = nc.inline_tensor(weights["G1ROW"], name="G1ROW")
        aff_c["B1L"] = nc.inline_tensor(weights["B1LROW"], name="B1LROW")
    if use_dn:
        aff_c["B1"] = nc.inline_tensor(weights["B1ROW"], name="B1ROW")
    if aff2:
        aff_c["GN"] = nc.inline_tensor(weights["GNROW"], name="GNROW")
        aff_c["BN"] = nc.inline_tensor(weights["BNROW"], name="BNROW")
    if use_b2:
        aff_c["B2"] = nc.inline_tensor(weights["B2ROW"], name="B2ROW")

    with tile.TileContext(nc) as tc:
        with tc.tile_pool(name="dram", bufs=1, space="DRAM") as dp, \
             tc.tile_pool(name="consts", bufs=1) as pcst:
            xsend_b = dp.tile([M, D], f8)
            tables_f8 = [dp.tile([Q, D], f8, addr_space="Shared",
                                 name=f"tablef8{q}") for q in range(4)]
            tables = [dp.tile([Q, D], bf, name=f"table{q}") for q in range(4)]

            # warm up the collective path (first collective pays ~40us of
            # staging) with a junk gather nobody reads
            cwarm_i = dp.tile([128, D], f8, name="cwarm_i")
            cwarm_o = dp.tile([NCORES * 128, D], f8, addr_space="Shared",
                              name="cwarm_o")
            nc.gpsimd.collective_compute(
                "AllGather", mybir.AluOpType.bypass,
                replica_groups=[list(range(NCORES))],
                ins=[cwarm_i[:, :].opt()], outs=[cwarm_o[:, :].opt()])

            S_sb = pcst.tile([128, P, 128], bf)
            nc.sync.dma_start(S_sb, S_c.ap().rearrange("(p d) e -> d p e", d=128))

            def emit_ag(q):
                nc.gpsimd.collective_compute(
                    "AllGather", mybir.AluOpType.bypass,
                    replica_groups=[list(range(NCORES))],
                    ins=[xsend_b[q * MQ:(q + 1) * MQ, :].opt()],
                    outs=[tables_f8[q].opt()])

            def emit_expand(q):
                # fp8 -> bf16 cast during DMA (SWDGE); the gather reads 256B
                # bf16 rows, the AllGather only moved 128B fp8 rows
                nc.gpsimd.dma_start(tables[q][:, :], tables_f8[q][:, :])

            # ---------------- Phase A: x_send + AllGather -------------------
            with tc.tile_pool(name="paX", bufs=1) as paX, \
                 tc.tile_pool(name="paE", bufs=4) as paE, \
                 tc.tile_pool(name="paP", bufs=4, space="PSUM") as paP:
                xT_sb = paX.tile([128, M], bf)
                for k in range(4):
                    nc.sync.dma_start(xT_sb[:, k * MQ:(k + 1) * MQ],
                                      xT_t.ap()[:, k * MQ:(k + 1) * MQ])
                blocks_per_q = W // 4
                for b in range(W):
                    ps = paP.tile([128, 128], f32, tag="xsps", name="ps")
                    nc.tensor.matmul(
                        ps, lhsT=xT_sb[:, b * 128:(b + 1) * 128],
                        rhs=S_sb[:, pol_of_block[b], :], start=True, stop=True)
                    ev = paE.tile([128, 128], f8, tag="xsev", name="ev")
                    nc.scalar.copy(ev, ps)
                    nc.sync.dma_start(xsend_b[b * 128:(b + 1) * 128, :], ev)
                    if (b + 1) % blocks_per_q == 0 and (b + 1) // blocks_per_q <= 2:
                        emit_ag((b + 1) // blocks_per_q - 1)

            # remaining constants (after Phase A so its DMAs queue first)
            idx_all = pcst.tile([128, EP // 16], i16)
            nc.sync.dma_start(idx_all, idx_t.ap())
            D_sb = pcst.tile([128, P, 128], bf)
            nc.sync.dma_start(D_sb, D_c.ap().rearrange("(p d) e -> d p e", d=128))
            W2_sb = pcst.tile([128, 128], bf)
            nc.sync.dma_start(W2_sb, W2_c.ap())
            id_sb = pcst.tile([128, 128], bf)
            nc.sync.dma_start(id_sb, ID_c.ap())
            eps_sb = pcst.tile([128, 1], f32)
            nc.vector.memset(eps_sb, LN_EPS)
            zero_sb = pcst.tile([128, GW], f32)
            nc.vector.memset(zero_sb, 0.0)
            dn_sb = None
            if use_dn:
                dn_sb = pcst.tile([128, W], f32)
                nc.sync.dma_start(dn_sb, dn_t.ap())
            aff_sb = {}
            for k, t in aff_c.items():
                aff_sb[k] = pcst.tile([128, 128], f32, name=f"aff_{k}")
                nc.sync.dma_start(aff_sb[k], t.ap())

            # ---------------- Phase B + C, fused per window-group -----------
            x_r = x_nm_t.ap().rearrange("(w p) f -> p w f", p=128)
            o_r = out_t.ap().rearrange("(w p) f -> p w f", p=128)

            with tc.tile_pool(name="pgG", bufs=GBUFS) as pgG, \
                 tc.tile_pool(name="pgH", bufs=3) as pgH, \
                 tc.tile_pool(name="pcc", bufs=4) as pcc, \
                 tc.tile_pool(name="pln", bufs=2) as pln, \
                 tc.tile_pool(name="psP", bufs=3, space="PSUM") as psP, \
                 tc.tile_pool(name="psZ", bufs=1, space="PSUM") as psZ, \
                 tc.tile_pool(name="psB", bufs=2, space="PSUM") as psB:
                state = {}
                ctr = dict(g=0)

                def open_group(gi):
                    gl = len(wgroups[gi])
                    nch_g = group_nch[gi]
                    pbig = psP.tile([128, GW * 128], f32, tag="spmm",
                                    name="pbig")[:, :gl * 128]
                    Hg = pgH.tile([128, NCHG, 128], f8, tag="H",
                                  name="Hg")[:, :nch_g, :]
                    nc.sync.dma_start(
                        Hg, hoh_t.ap()[:, group_ch0[gi]:group_ch0[gi] + nch_g, :])
                    state[gi] = (pbig, Hg)

                def emit_batches(gi, qs):
                    pbig, Hg = state[gi]
                    w0 = wgroups[gi][0]
                    gch0 = group_ch0[gi]
                    for bi, (q, ch0, ch1) in enumerate(batches_by_group[gi]):
                        if q not in qs:
                            continue
                        nch_b = ch1 - ch0
                        G = pgG.tile([128, SUBB, 128], bf, tag="G",
                                     name="G")[:, :nch_b, :]
                        nc.gpsimd.dma_gather(
                            out_ap=G, in_ap=tables[q][:, :],
                            idxs_ap=idx_all[:, ch0 * 8:ch1 * 8],
                            num_idxs=nch_b * 128,
                            num_idxs_reg=batch_cnt_by_group[gi][bi],
                            elem_size=128, queue_num=ctr["g"] % 4)
                        ctr["g"] += 1
                        for j in range(ch0, ch1):
                            w, k = chunk_w[j], chunk_k[j]
                            i = w - w0
                            # PSUM start=True clears the WHOLE bank — emit it
                            # only on the chronologically-first matmul into
                            # each bank; later slices then overwrite via
                            # cleared has_written bits and accumulate after.
                            nc.tensor.matmul(
                                pbig[:, i * 128:(i + 1) * 128],
                                lhsT=G[:, j - ch0, :], rhs=Hg[:, j - gch0, :],
                                start=(q == 0 and k == 0 and i == 0),
                                stop=(q == 3 and k == C[w][3] - 1),
                                skip_group_check=True)

                def emit_phase_c(gi):
                    pbig, _ = state.pop(gi)
                    wg = wgroups[gi]
                    gl = len(wg)
                    w0 = wg[0]
                    aggT = pcc.tile([128, GW * 128], bf, tag="aggT",
                                    name="aggT")[:, :gl * 128]
                    nc.scalar.copy(aggT, pbig)

                    # z1 = aggT.T @ D_p  ->  [dest(part), feat], one PSUM bank
                    psz = psZ.tile([128, GW * 128], f32, tag="psz",
                                   name="psz")[:, :gl * 128]
                    for i, w in enumerate(wg):
                        nc.tensor.matmul(
                            psz[:, i * 128:(i + 1) * 128],
                            lhsT=aggT[:, i * 128:(i + 1) * 128],
                            rhs=D_sb[:, pol_of_block[w], :],
                            start=(i == 0), stop=True, skip_group_check=True)

                    z1v = psz
                    if use_dn or aff1:
                        # fallback: materialize z1 in SBUF to apply dn/b1
                        z1f = pcc.tile([128, GW * 128], f32, tag="z1f",
                                       name="z1f")[:, :gl * 128]
                        for i, w in enumerate(wg):
                            sc = dn_sb[:, w:w + 1] if use_dn else 1.0
                            nc.scalar.activation(
                                z1f[:, i * 128:(i + 1) * 128],
                                psz[:, i * 128:(i + 1) * 128],
                                AF.Identity, scale=sc)
                        if use_dn:
                            z3 = z1f.rearrange("p (w f) -> p w f", f=128)
                            nc.vector.tensor_tensor(
                                z3, z3,
                                aff_sb["B1"][:, None, :]
                                .to_broadcast([128, gl, 128]),
                                op=mybir.AluOpType.add)
                        z1v = z1f

                    # LayerNorm 1 stats
                    stats = pln.tile([128, GW, 6], f32, tag="bnst",
                                     name="stats")[:, :gl, :]
                    for i in range(gl):
                        nc.vector.bn_stats(stats[:, i, :],
                                           z1v[:, i * 128:(i + 1) * 128])
                    mv = pln.tile([128, GW, 2], f32, tag="bnmv",
                                  name="mv")[:, :gl, :]
                    for i in range(gl):
                        nc.vector.bn_aggr(mv[:, i, :], stats[:, i, :])
                    rstd = pln.tile([128, GW], f32, tag="rstd",
                                    name="rstd")[:, :gl]
                    nc.scalar.activation(rstd, mv[:, :, 1], AF.Sqrt,
                                         bias=eps_sb[:, 0:1])
                    nc.vector.reciprocal(rstd, rstd)
                    nmr = pln.tile([128, GW], f32, tag="nmr",
                                   name="nmr")[:, :gl]
                    nc.vector.tensor_tensor(nmr, mv[:, :, 0], rstd,
                                            op=mybir.AluOpType.mult)
                    nc.vector.tensor_tensor(nmr, zero_sb[:, :gl], nmr,
                                            op=mybir.AluOpType.subtract)

                    # apply LN1 (+ReLU) on the scalar engine: relu(z*r - mu*r)
                    h = pcc.tile([128, GW * 128], bf, tag="h",
                                 name="h")[:, :gl * 128]
                    fn1 = AF.Identity if aff1 else AF.Relu
                    for i in range(gl):
                        nc.scalar.activation(
                            h[:, i * 128:(i + 1) * 128],
                            z1v[:, i * 128:(i + 1) * 128], fn1,
                            scale=rstd[:, i:i + 1], bias=nmr[:, i:i + 1])
                    if aff1:
                        h3 = h.rearrange("p (w f) -> p w f", f=128)
                        nc.vector.tensor_tensor(
                            h3, h3,
                            aff_sb["G1"][:, None, :].to_broadcast([128, gl, 128]),
                            op=mybir.AluOpType.mult)
                        nc.vector.tensor_tensor(
                            h3, h3,
                            aff_sb["B1L"][:, None, :].to_broadcast([128, gl, 128]),
                            op=mybir.AluOpType.add)
                        nc.scalar.activation(h, h, AF.Relu)

                    # transpose h, apply W2, residual
                    hT = pcc.tile([128, GW * 128], bf, tag="hT",
                                  name="hT")[:, :gl * 128]
                    for i in range(gl):
                        pst = psB.tile([128, 128], bf, tag="pst", name="pst")
                        nc.tensor.transpose(
                            pst, h[:, i * 128:(i + 1) * 128], id_sb)
                        nc.scalar.copy(hT[:, i * 128:(i + 1) * 128], pst)
                    xg = pcc.tile([128, GW * 128], f32, tag="xg",
                                  name="xg")[:, :gl * 128]
                    nc.sync.dma_start(
                        xg.rearrange("p (w f) -> p w f", f=128),
                        x_r[:, w0:w0 + gl, :])
                    og = pcc.tile([128, GW * 128], f32, tag="og",
                                  name="og")[:, :gl * 128]
                    for i in range(gl):
                        ps5 = psB.tile([128, 128], f32, tag="ps5", name="ps5")
                        nc.tensor.matmul(ps5,
                                         lhsT=hT[:, i * 128:(i + 1) * 128],
                                         rhs=W2_sb, start=True, stop=True)
                        nc.vector.tensor_add(og[:, i * 128:(i + 1) * 128],
                                             ps5, xg[:, i * 128:(i + 1) * 128])
                    og_3d = og.rearrange("p (w f) -> p w f", f=128)
                    if use_b2:
                        nc.vector.tensor_tensor(
                            og_3d, og_3d,
                            aff_sb["B2"][:, None, :].to_broadcast([128, gl, 128]),
                            op=mybir.AluOpType.add)

                    # LayerNorm 2
                    stats2 = pln.tile([128, GW, 6], f32, tag="bnst",
                                      name="stats2")[:, :gl, :]
                    for i in range(gl):
                        nc.vector.bn_stats(stats2[:, i, :],
                                           og[:, i * 128:(i + 1) * 128])
                    mv2 = pln.tile([128, GW, 2], f32, tag="bnmv",
                                   name="mv2")[:, :gl, :]
                    for i in range(gl):
                        nc.vector.bn_aggr(mv2[:, i, :], stats2[:, i, :])
                    rstd2 = pln.tile([128, GW], f32, tag="rstd",
                                     name="rstd2")[:, :gl]
                    nc.scalar.activation(rstd2, mv2[:, :, 1], AF.Sqrt,
                                         bias=eps_sb[:, 0:1])
                    nc.vector.reciprocal(rstd2, rstd2)
                    nmr2 = pln.tile([128, GW], f32, tag="nmr",
                                    name="nmr2")[:, :gl]
                    nc.vector.tensor_tensor(nmr2, mv2[:, :, 0], rstd2,
                                            op=mybir.AluOpType.mult)
                    nc.vector.tensor_tensor(nmr2, zero_sb[:, :gl], nmr2,
                                            op=mybir.AluOpType.subtract)

                    outg = pcc.tile([128, GW * 128], f32, tag="outg",
                                    name="outg")[:, :gl * 128]
                    for i in range(gl):
                        nc.scalar.activation(
                            outg[:, i * 128:(i + 1) * 128],
                            og[:, i * 128:(i + 1) * 128], AF.Identity,
                            scale=rstd2[:, i:i + 1], bias=nmr2[:, i:i + 1])
                    outg_3d = outg.rearrange("p (w f) -> p w f", f=128)
                    if aff2:
                        nc.vector.tensor_tensor(
                            outg_3d, outg_3d,
                            aff_sb["GN"][:, None, :].to_broadcast([128, gl, 128]),
                            op=mybir.AluOpType.mult)
                        nc.vector.tensor_tensor(
                            outg_3d, outg_3d,
                            aff_sb["BN"][:, None, :].to_broadcast([128, gl, 128]),
                            op=mybir.AluOpType.add)

                    nc.sync.dma_start(o_r[:, w0:w0 + gl, :], outg_3d)

                # warm up the Q7 gather ucode (first dma_gather pays a ~6us
                # IRAM load): junk 128-idx gather nobody reads
                widx = pcst.tile([128, 8], i16)
                nc.vector.memset(widx, 0)
                wG = pgG.tile([128, 1, 128], f32, tag="warmG", name="wG")
                nc.gpsimd.dma_gather(
                    out_ap=wG, in_ap=x_nm_t.ap()[:, :], idxs_ap=widx,
                    num_idxs=128, num_idxs_reg=128, elem_size=128,
                    queue_num=3)

                # Warm-up schedule: lead with gathers for ready tables while
                # the AllGather chain (serialized on the collective queue)
                # drains; AG triggers and table expands are interleaved so the
                # Pool engine never parks long on a collective-queue slot.
                open_group(0)
                open_group(1)
                open_group(2)
                emit_expand(0)
                emit_batches(0, (0,))
                emit_batches(1, (0,))
                emit_batches(2, (0,))
                emit_ag(2)
                emit_expand(1)
                emit_batches(0, (1,))
                emit_batches(1, (1,))
                emit_batches(2, (1,))
                emit_ag(3)
                emit_expand(2)
                for gi in (0, 1, 2):
                    emit_batches(gi, (2,))
                emit_expand(3)
                for gi in (0, 1, 2):
                    emit_batches(gi, (3,))
                    emit_phase_c(gi)
                for gi in range(3, NG):
                    open_group(gi)
                    emit_batches(gi, (0, 1, 2, 3))
                    emit_phase_c(gi)

    nc.compile()
    return nc


# ----------------------------------------------------------------------------
# entry points
# ----------------------------------------------------------------------------

def _assemble(results_list, perm, N, D):
    out = np.empty((N, D), np.float32)
    pc = perm.reshape(NCORES, -1)
    for c in range(NCORES):
        m = pc[c] >= 0
        out[pc[c][m]] = results_list[c][m]
    return out


def _install_ntff_hook_shim():
    """This image's antenv lacks axon_hooks; synthesize it so trace=True can
    reach the libaxon NTFF profiler (see trn_agent_boot.trn_boot)."""
    import types
    if "antenv.axon_hooks" in sys.modules:
        return
    try:
        from trn_agent_boot.trn_boot import _ntff_profile_via_ctypes
        hook = _ntff_profile_via_ctypes("/opt/axon/libaxon_pjrt.so")
    except Exception:
        hook = None
    mod = types.ModuleType("antenv.axon_hooks")
    state = {"hook": hook}
    mod.get_axon_ntff_profile_hook = lambda: state["hook"]
    mod.set_axon_ntff_profile_hook = lambda h: state.update(hook=h)
    sys.modules["antenv.axon_hooks"] = mod


def _run_hw(nc, in_maps, trace=False):
    if trace:
        sys.path.insert(0, "/root/.axon_site")
        _install_ntff_hook_shim()
    from concourse.bass_utils import run_bass_kernel_spmd
    res = run_bass_kernel_spmd(nc, in_maps, core_ids=list(range(NCORES)),
                               trace=trace)
    return res


def _run_sim(nc, in_maps):
    from concourse.bass_interp import MultiCoreSim
    sim = MultiCoreSim(nc, num_cores=NCORES, trace=False,
                       require_finite=False, require_nnan=False)
    cores = list(sim.cores.values())
    for c, core in enumerate(cores):
        for k, v in in_maps[c].items():
            core.tensor(k)[:] = v
    sim.simulate(check_with_hw=False)
    return [np.array(core.tensor("out")) for core in cores]


def kernel(**inputs) -> np.ndarray:
    cfg, weights, in_maps, perm, N = _prepare(inputs)
    nc = _build_nc(cfg, weights)
    res = _run_hw(nc, in_maps)
    outs = [res.results[c]["out"] for c in range(NCORES)]
    return _assemble(outs, perm, N, cfg["D"])


# revision 21
# speedup vs baseline: 1.0385x; 1.0385x over previous
"""Trainium2 Bass kernel for MinimalCopresheafTNN (GNN message passing).

Strategy (8 NeuronCores, SPMD single program):
  * Host: fold W_r / R[p] / W1 into one per-polarity matrix D_p = W_r.T @ R_p @ W1.T
    (linearity of segment_sum), fold res_scale into W2. Since b1 == 0, the
    per-node deg_norm scale cancels through LayerNorm1 (LN is invariant to
    positive per-row scaling), so it is dropped entirely. Permute nodes so
    each core owns a contiguous, polarity-grouped slice (segments padded to
    128 and uniform across cores).
  * Device, per core:
      Phase A: x_send = x @ S[pol] for the core's slice (bf16 matmuls),
               AllGather the full x_send table into HBM on every core.
      Phase B: SpMM agg = scatter-add of x_send[row] into the core's dest
               nodes: dma_gather of source rows (4 table quadrants, int16
               indices, max 1024/call) + one-hot matmul scatter into PSUM
               per 128-dest window.
      Phase C: z1 = aggT @ D_p -> LayerNorm(+ReLU on scalar engine) ->
               transpose -> @ (res*W2.T) + x -> LayerNorm -> out.
  * Host: inverse-permute per-core outputs into the full [N, D] result.
"""

import sys

import numpy as np

sys.path.insert(0, "/opt/trn_rl_repo")

NCORES = 8
LN_EPS = 1e-5
DMA_SCRATCH = 65536
GW = 4                           # windows per group (PSUM bank = 512 f32)
SUBB = 8                         # chunks per gather batch (1024 idx HW limit)
GBUFS = 12                       # G-tile pipeline depth


# ----------------------------------------------------------------------------
# host-side preparation
# ----------------------------------------------------------------------------

def _prepare(inputs):
    x = np.asarray(inputs["x"], np.float32)
    N, D = x.shape
    S = (np.asarray(inputs["send_maps"], np.float32)
         + np.asarray(inputs["delta_send"], np.float32))
    Rm = (np.asarray(inputs["receive_maps"], np.float32)
          + np.asarray(inputs["delta_receive"], np.float32))
    P = S.shape[0]
    W_r = np.asarray(inputs["W_r"], np.float32)
    W1 = np.asarray(inputs["W1"], np.float32)
    b1 = np.asarray(inputs["b1"], np.float32)
    ln1_g = np.asarray(inputs["ln1_g"], np.float32)
    ln1_b = np.asarray(inputs["ln1_b"], np.float32)
    W2 = np.asarray(inputs["W2"], np.float32)
    b2 = np.asarray(inputs["b2"], np.float32)
    norm_g = np.asarray(inputs["norm_g"], np.float32)
    norm_b = np.asarray(inputs["norm_b"], np.float32)
    res = float(np.asarray(inputs["res_scale"]))
    row = np.asarray(inputs["row"]).astype(np.int64)
    col = np.asarray(inputs["col"]).astype(np.int64)
    pols = np.asarray(inputs["ring_polarities"]).astype(np.int64) % P
    E = row.shape[0]

    deg = np.bincount(row, minlength=N).astype(np.float32)
    dn = (1.0 / np.maximum(deg, 1.0)).astype(np.float32)
    indeg = np.bincount(col, minlength=N)

    # deg_norm is a positive per-row scale applied before `@ W1 + b1` and
    # LayerNorm; when b1 == 0 LayerNorm's scale invariance cancels it.
    use_dn = not bool(np.all(b1 == 0))
    trivial_aff1 = bool(np.all(ln1_g == 1) and np.all(ln1_b == 0))
    trivial_aff2 = bool(np.all(norm_g == 1) and np.all(norm_b == 0))
    trivial_b2 = bool(np.all(b2 == 0))

    # --- node -> (core, position) assignment --------------------------------
    L = np.zeros(P, np.int64)              # padded segment length per polarity
    core_nodes = [[None] * P for _ in range(NCORES)]
    for p in range(P):
        nodes_p = np.where(pols == p)[0]
        order = nodes_p[np.argsort(-indeg[nodes_p], kind="stable")]
        mx = 0
        for c in range(NCORES):
            core_nodes[c][p] = order[c::NCORES]
            mx = max(mx, len(core_nodes[c][p]))
        L[p] = max(128, ((mx + 127) // 128) * 128)
    M = int(L.sum())
    M = ((M + 511) // 512) * 512          # quadrants must be block-aligned
    W = M // 128
    NP = NCORES * M
    MQ = M // 4
    Q = NCORES * MQ                       # rows per quadrant table
    assert Q <= 32767, f"quadrant rows {Q} exceed int16 range"

    seg_start = np.concatenate([[0], np.cumsum(L)[:-1]])
    pol_of_block = np.repeat(np.arange(P), L // 128)
    pol_of_block = np.concatenate(
        [pol_of_block, np.full(W - len(pol_of_block), P - 1, np.int64)])

    perm = np.full(NP, -1, dtype=np.int64)
    for c in range(NCORES):
        for p in range(P):
            nodes = core_nodes[c][p]
            n_w = L[p] // 128
            base = c * M + seg_start[p]
            j = np.arange(len(nodes))
            perm[base + (j % n_w) * 128 + j // n_w] = nodes
    real = perm >= 0
    pos_of = np.empty(N, dtype=np.int64)
    pos_of[perm[real]] = np.nonzero(real)[0]

    # --- edge layout --------------------------------------------------------
    col_pos = pos_of[col]
    row_pos = pos_of[row]
    core_e = col_pos // M
    w_e = (col_pos % M) // 128
    rel_e = (col_pos % M) % 128
    n_in_core = row_pos % M
    q_e = n_in_core // MQ
    rel_s = (row_pos // M) * MQ + (n_in_core % MQ)
    dn_e = dn[col]

    key = (core_e * W + w_e) * 4 + q_e
    cnt = np.bincount(key, minlength=NCORES * W * 4).reshape(NCORES, W, 4)
    C = np.maximum(1, -(-cnt.max(axis=0) // 128)).astype(np.int64)      # [W, 4]

    wgroups = [list(range(g, min(g + GW, W))) for g in range(0, W, GW)]

    chunk_start = np.zeros((W, 4), np.int64)
    chunk_w, chunk_q, chunk_k = [], [], []
    batches_by_group = []            # [gi] -> list of (q, ch0, ch1), <= SUBB
    group_ch0 = []                   # first chunk id of each group
    nch = 0
    for wg in wgroups:
        group_ch0.append(nch)
        gb = []
        for q in range(4):
            b0 = nch
            for w in wg:
                chunk_start[w, q] = nch
                for k in range(C[w, q]):
                    chunk_w.append(w)
                    chunk_q.append(q)
                    chunk_k.append(k)
                nch += C[w, q]
            for s0 in range(b0, nch, SUBB):
                gb.append((q, s0, min(s0 + SUBB, nch)))
        batches_by_group.append(gb)
    NCH = int(nch)
    EP = 128 * NCH
    group_nch = [(batches_by_group[gi][-1][2] - group_ch0[gi])
                 for gi in range(len(wgroups))]
    NCHG = max(group_nch)

    import ml_dtypes
    bf16 = ml_dtypes.bfloat16
    f8 = ml_dtypes.float8_e4m3
    idx_arr = np.zeros((NCORES, EP), np.int16)
    reld_arr = np.full((NCORES, 128, NCH), -1, np.int16)

    order_e = np.argsort(key, kind="stable")
    counts_flat = np.bincount(key, minlength=NCORES * W * 4)
    group_start = np.zeros(NCORES * W * 4 + 1, np.int64)
    group_start[1:] = np.cumsum(counts_flat)
    r = np.arange(E) - group_start[key[order_e]]
    c_of = core_e[order_e]
    tchunk = chunk_start[w_e[order_e], q_e[order_e]] + r // 128
    lane = r % 128
    s = tchunk * 128 + lane
    idx_arr[c_of, s] = rel_s[order_e].astype(np.int16)
    reld_arr[c_of, lane, tchunk] = rel_e[order_e].astype(np.int16)

    # Uniform trailing trim: the Q7 gather ucode skips trailing negative
    # indices, and num_idxs_reg must equal the non-negative count — trim
    # every batch at the max-over-cores last-real-edge position (identical on
    # all cores). First GBUFS batches untouched (first-use G slots may be NaN).
    occupied = np.zeros((NCORES, EP), bool)
    occupied[c_of, s] = True
    batch_cnt_by_group = []
    for gi, gb in enumerate(batches_by_group):
        cnts = []
        for (_, ch0, ch1) in gb:
            Lb = (ch1 - ch0) * 128
            # first-emitted groups left untrimmed (first-use G slots are NaN)
            if gi < 3:
                cnts.append(Lb)
                continue
            nz = np.nonzero(occupied[:, ch0 * 128:ch1 * 128].any(axis=0))[0]
            T = int(nz[-1] + 1) if len(nz) else 16
            T = min(Lb, ((T + 15) // 16) * 16)
            idx_arr[:, ch0 * 128 + T:ch1 * 128] = -1
            cnts.append(T)
        batch_cnt_by_group.append(cnts)

    # host-built one-hot scatter matrix (0/1 exact in fp8): [128, NCH, 128]
    hoh = (reld_arr[:, :, :, None]
           == np.arange(128, dtype=np.int16)[None, None, None, :]).astype(f8)

    # wrapped + replicated gather-index layout: idx i lives at [i%16, i//16],
    # replicated over the 8 Q7 partition groups
    idx_rep = np.empty((NCORES, 128, EP // 16), np.int16)
    for c in range(NCORES):
        idx_rep[c] = np.tile(idx_arr[c].reshape(EP // 16, 16).T, (8, 1))

    # --- per-core node data -------------------------------------------------
    x_nm = np.zeros((NCORES, M, D), np.float32)
    pc = perm.reshape(NCORES, M)
    for c in range(NCORES):
        m = pc[c] >= 0
        x_nm[c][m] = x[pc[c][m]]
    xT = np.ascontiguousarray(x_nm.transpose(0, 2, 1)).astype(bf16)

    # per-window deg_norm column (only used when b1 != 0)
    dn_nm = np.ones((NCORES, M), np.float32)
    for c in range(NCORES):
        m = pc[c] >= 0
        dn_nm[c][m] = dn[pc[c][m]]
    dn_cols = dn_nm.reshape(NCORES, W, 128).transpose(0, 2, 1).copy()

    # --- fused weights ------------------------------------------------------
    D_all = np.einsum(
        "de,pef,fg->pdg",
        W_r.T.astype(np.float64), Rm.astype(np.float64), W1.T.astype(np.float64),
    ).astype(np.float32)
    W2s = (res * W2.T).astype(np.float32)

    cfg = dict(
        D=D, P=P, M=M, W=W, NP=NP, Q=Q, MQ=MQ, NCH=NCH, EP=EP, NCHG=NCHG,
        pol_of_block=pol_of_block.tolist(),
        wgroups=wgroups, C=C, batches_by_group=batches_by_group,
        batch_cnt_by_group=batch_cnt_by_group, group_ch0=group_ch0,
        group_nch=group_nch,
        chunk_w=chunk_w, chunk_k=chunk_k,
        use_dn=use_dn, trivial_aff1=trivial_aff1, trivial_aff2=trivial_aff2,
        trivial_b2=trivial_b2,
    )
    weights = dict(
        S_all=np.ascontiguousarray(S.reshape(P * D, D)).astype(bf16),
        D_all=np.ascontiguousarray(D_all.reshape(P * D, D)).astype(bf16),
        W2s=np.ascontiguousarray(W2s).astype(bf16),
        IDENT=np.eye(128, dtype=np.float32).astype(bf16),
        B1ROW=np.tile(b1, (128, 1)).astype(np.float32),
        G1ROW=np.tile(ln1_g, (128, 1)).astype(np.float32),
        B1LROW=np.tile(ln1_b, (128, 1)).astype(np.float32),
        GNROW=np.tile(norm_g, (128, 1)).astype(np.float32),
        BNROW=np.tile(norm_b, (128, 1)).astype(np.float32),
        B2ROW=np.tile(res * b2, (128, 1)).astype(np.float32),
    )
    in_maps = [
        dict(x_nm=x_nm[c], xT=xT[c], idx=idx_rep[c], hoh=hoh[c],
             dncol=dn_cols[c])
        for c in range(NCORES)
    ]
    return cfg, weights, in_maps, perm, N


# ----------------------------------------------------------------------------
# device program
# ----------------------------------------------------------------------------

def _build_nc(cfg, weights):
    import concourse.bass as bass
    import concourse.mybir as mybir
    import concourse.tile as tile
    from concourse import bacc

    f32 = mybir.dt.float32
    bf = mybir.dt.bfloat16
    f8 = mybir.dt.float8e4
    i16 = mybir.dt.int16
    AF = mybir.ActivationFunctionType
    D, P, M, W = cfg["D"], cfg["P"], cfg["M"], cfg["W"]
    NP, Q, NCH, EP = cfg["NP"], cfg["Q"], cfg["NCH"], cfg["EP"]
    MQ, NCHG = cfg["MQ"], cfg["NCHG"]
    pol_of_block = cfg["pol_of_block"]
    wgroups, C = cfg["wgroups"], cfg["C"]
    batches_by_group = cfg["batches_by_group"]
    batch_cnt_by_group = cfg["batch_cnt_by_group"]
    group_ch0, group_nch = cfg["group_ch0"], cfg["group_nch"]
    chunk_w, chunk_k = cfg["chunk_w"], cfg["chunk_k"]
    use_dn = cfg["use_dn"]
    aff1, aff2 = not cfg["trivial_aff1"], not cfg["trivial_aff2"]
    use_b2 = not cfg["trivial_b2"]
    NG = len(wgroups)

    nc = bacc.Bacc("TRN2", target_bir_lowering=False, debug=False,
                   num_devices=NCORES, enable_asserts=False,
                   dynamic_dma_scratch_size=DMA_SCRATCH,
                   num_swdge_queues=4)

    x_nm_t = nc.dram_tensor("x_nm", [M, D], f32, kind="ExternalInput")
    xT_t = nc.dram_tensor("xT", [D, M], bf, kind="ExternalInput")
    idx_t = nc.dram_tensor("idx", [128, EP // 16], i16, kind="ExternalInput")
    hoh_t = nc.dram_tensor("hoh", [128, NCH, 128], f8, kind="ExternalInput")
    dn_t = nc.dram_tensor("dncol", [128, W], f32, kind="ExternalInput")
    out_t = nc.dram_tensor("out", [M, D], f32, kind="ExternalOutput")

    S_c = nc.inline_tensor(weights["S_all"], name="S_all")
    D_c = nc.inline_tensor(weights["D_all"], name="D_all")
    W2_c = nc.inline_tensor(weights["W2s"], name="W2s")
    ID_c = nc.inline_tensor(weights["IDENT"], name="IDENT")
    aff_c = {}
    if aff1:
        aff_c["G1"] = nc.inline_tensor(weights["G1ROW"], name="G1ROW")
        aff_c["B1L"] = nc.inline_tensor(weights["B1LROW"], name="B1LROW")
    if use_dn:
        aff_c["B1"] = nc.inline_tensor(weights["B1ROW"], name="B1ROW")
    if aff2:
        aff_c["GN"] = nc.inline_tensor(weights["GNROW"], name="GNROW")
        aff_c["BN"] = nc.inline_tensor(weights["BNROW"], name="BNROW")
    if use_b2:
        aff_c["B2"] = nc.inline_tensor(weights["B2ROW"], name="B2ROW")

    with tile.TileContext(nc) as tc:
        with tc.tile_pool(name="dram", bufs=1, space="DRAM") as dp, \
             tc.tile_pool(name="consts", bufs=1) as pcst:
            xsend_b = dp.tile([M, D], f8)
            tables_f8 = [dp.tile([Q, D], f8, addr_space="Shared",
                                 name=f"tablef8{q}") for q in range(4)]
            tables = [dp.tile([Q, D], bf, name=f"table{q}") for q in range(4)]

            S_sb = pcst.tile([128, P, 128], bf)
            nc.sync.dma_start(S_sb, S_c.ap().rearrange("(p d) e -> d p e", d=128))

            def emit_ag(q):
                nc.gpsimd.collective_compute(
                    "AllGather", mybir.AluOpType.bypass,
                    replica_groups=[list(range(NCORES))],
                    ins=[xsend_b[q * MQ:(q + 1) * MQ, :].opt()],
                    outs=[tables_f8[q].opt()])

            A_ROWS = Q // 128           # table rows per partition stripe
            EXS = 8                     # expansion sub-chunks per quadrant
            AS = A_ROWS // EXS

            def make_expand(pex):
                f8_r = [t[:, :].rearrange("(p a) c -> p a c", p=128)
                        for t in tables_f8]
                bf_r = [t[:, :].rearrange("(p a) c -> p a c", p=128)
                        for t in tables]

                def emit_expand(q):
                    # fp8 -> bf16 row expansion routed through SBUF: the
                    # AllGather moves 128B fp8 rows, the gather needs 256B
                    # bf16 rows. Contiguous per-partition stripes keep the
                    # DMA at 128 fat descriptors each way.
                    for s in range(EXS):
                        fin = pex.tile([128, AS, 128], f8, tag="exf",
                                       name="fin")
                        nc.sync.dma_start(
                            fin, f8_r[q][:, s * AS:(s + 1) * AS, :])
                        fout = pex.tile([128, AS, 128], bf, tag="exo",
                                        name="fout")
                        nc.vector.tensor_copy(fout, fin)
                        nc.sync.dma_start(
                            bf_r[q][:, s * AS:(s + 1) * AS, :], fout)
                return emit_expand

            # ---------------- Phase A: x_send + AllGather -------------------
            with tc.tile_pool(name="paX", bufs=1) as paX, \
                 tc.tile_pool(name="paE", bufs=4) as paE, \
                 tc.tile_pool(name="paP", bufs=4, space="PSUM") as paP:
                xT_sb = paX.tile([128, M], bf)
                for k in range(4):
                    nc.sync.dma_start(xT_sb[:, k * MQ:(k + 1) * MQ],
                                      xT_t.ap()[:, k * MQ:(k + 1) * MQ])
                blocks_per_q = W // 4
                for b in range(W):
                    ps = paP.tile([128, 128], f32, tag="xsps", name="ps")
                    nc.tensor.matmul(
                        ps, lhsT=xT_sb[:, b * 128:(b + 1) * 128],
                        rhs=S_sb[:, pol_of_block[b], :], start=True, stop=True)
                    ev = paE.tile([128, 128], f8, tag="xsev", name="ev")
                    nc.scalar.copy(ev, ps)
                    nc.sync.dma_start(xsend_b[b * 128:(b + 1) * 128, :], ev)
                    if (b + 1) % blocks_per_q == 0 and (b + 1) // blocks_per_q <= 2:
                        emit_ag((b + 1) // blocks_per_q - 1)

            # remaining constants (after Phase A so its DMAs queue first)
            idx_all = pcst.tile([128, EP // 16], i16)
            nc.sync.dma_start(idx_all, idx_t.ap())
            D_sb = pcst.tile([128, P, 128], bf)
            nc.sync.dma_start(D_sb, D_c.ap().rearrange("(p d) e -> d p e", d=128))
            W2_sb = pcst.tile([128, 128], bf)
            nc.sync.dma_start(W2_sb, W2_c.ap())
            id_sb = pcst.tile([128, 128], bf)
            nc.sync.dma_start(id_sb, ID_c.ap())
            eps_sb = pcst.tile([128, 1], f32)
            nc.vector.memset(eps_sb, LN_EPS)
            zero_sb = pcst.tile([128, GW], f32)
            nc.vector.memset(zero_sb, 0.0)
            dn_sb = None
            if use_dn:
                dn_sb = pcst.tile([128, W], f32)
                nc.sync.dma_start(dn_sb, dn_t.ap())
            aff_sb = {}
            for k, t in aff_c.items():
                aff_sb[k] = pcst.tile([128, 128], f32, name=f"aff_{k}")
                nc.sync.dma_start(aff_sb[k], t.ap())

            # ---------------- Phase B + C, fused per window-group -----------
            x_r = x_nm_t.ap().rearrange("(w p) f -> p w f", p=128)
            o_r = out_t.ap().rearrange("(w p) f -> p w f", p=128)

            with tc.tile_pool(name="pgG", bufs=GBUFS) as pgG, \
                 tc.tile_pool(name="pgH", bufs=3) as pgH, \
                 tc.tile_pool(name="pcc", bufs=4) as pcc, \
                 tc.tile_pool(name="pln", bufs=2) as pln, \
                 tc.tile_pool(name="pex", bufs=2) as pex, \
                 tc.tile_pool(name="psP", bufs=3, space="PSUM") as psP, \
                 tc.tile_pool(name="psZ", bufs=1, space="PSUM") as psZ, \
                 tc.tile_pool(name="psB", bufs=2, space="PSUM") as psB:
                emit_expand = make_expand(pex)
                state = {}
                ctr = dict(g=0)

                def open_group(gi):
                    gl = len(wgroups[gi])
                    nch_g = group_nch[gi]
                    pbig = psP.tile([128, GW * 128], f32, tag="spmm",
                                    name="pbig")[:, :gl * 128]
                    Hg = pgH.tile([128, NCHG, 128], f8, tag="H",
                                  name="Hg")[:, :nch_g, :]
                    nc.sync.dma_start(
                        Hg, hoh_t.ap()[:, group_ch0[gi]:group_ch0[gi] + nch_g, :])
                    state[gi] = (pbig, Hg)

                def emit_batches(gi, qs):
                    pbig, Hg = state[gi]
                    w0 = wgroups[gi][0]
                    gch0 = group_ch0[gi]
                    for bi, (q, ch0, ch1) in enumerate(batches_by_group[gi]):
                        if q not in qs:
                            continue
                        nch_b = ch1 - ch0
                        G = pgG.tile([128, SUBB, 128], bf, tag="G",
                                     name="G")[:, :nch_b, :]
                        nc.gpsimd.dma_gather(
                            out_ap=G, in_ap=tables[q][:, :],
                            idxs_ap=idx_all[:, ch0 * 8:ch1 * 8],
                            num_idxs=nch_b * 128,
                            num_idxs_reg=batch_cnt_by_group[gi][bi],
                            elem_size=128, queue_num=ctr["g"] % 4)
                        ctr["g"] += 1
                        for j in range(ch0, ch1):
                            w, k = chunk_w[j], chunk_k[j]
                            i = w - w0
                            # PSUM start=True clears the WHOLE bank — emit it
                            # only on the chronologically-first matmul into
                            # each bank; later slices then overwrite via
                            # cleared has_written bits and accumulate after.
                            nc.tensor.matmul(
                                pbig[:, i * 128:(i + 1) * 128],
                                lhsT=G[:, j - ch0, :], rhs=Hg[:, j - gch0, :],
                                start=(q == 0 and k == 0 and i == 0),
                                stop=(q == 3 and k == C[w][3] - 1),
                                skip_group_check=True)

                def emit_phase_c(gi):
                    pbig, _ = state.pop(gi)
                    wg = wgroups[gi]
                    gl = len(wg)
                    w0 = wg[0]
                    aggT = pcc.tile([128, GW * 128], bf, tag="aggT",
                                    name="aggT")[:, :gl * 128]
                    nc.scalar.copy(aggT, pbig)

                    # z1 = aggT.T @ D_p  ->  [dest(part), feat], one PSUM bank
                    psz = psZ.tile([128, GW * 128], f32, tag="psz",
                                   name="psz")[:, :gl * 128]
                    for i, w in enumerate(wg):
                        nc.tensor.matmul(
                            psz[:, i * 128:(i + 1) * 128],
                            lhsT=aggT[:, i * 128:(i + 1) * 128],
                            rhs=D_sb[:, pol_of_block[w], :],
                            start=(i == 0), stop=True, skip_group_check=True)

                    z1v = psz
                    if use_dn or aff1:
                        # fallback: materialize z1 in SBUF to apply dn/b1
                        z1f = pcc.tile([128, GW * 128], f32, tag="z1f",
                                       name="z1f")[:, :gl * 128]
                        for i, w in enumerate(wg):
                            sc = dn_sb[:, w:w + 1] if use_dn else 1.0
                            nc.scalar.activation(
                                z1f[:, i * 128:(i + 1) * 128],
                                psz[:, i * 128:(i + 1) * 128],
                                AF.Identity, scale=sc)
                        if use_dn:
                            z3 = z1f.rearrange("p (w f) -> p w f", f=128)
                            nc.vector.tensor_tensor(
                                z3, z3,
                                aff_sb["B1"][:, None, :]
                                .to_broadcast([128, gl, 128]),
                                op=mybir.AluOpType.add)
                        z1v = z1f

                    # LayerNorm 1 stats
                    stats = pln.tile([128, GW, 6], f32, tag="bnst",
                                     name="stats")[:, :gl, :]
                    for i in range(gl):
                        nc.vector.bn_stats(stats[:, i, :],
                                           z1v[:, i * 128:(i + 1) * 128])
                    mv = pln.tile([128, GW, 2], f32, tag="bnmv",
                                  name="mv")[:, :gl, :]
                    for i in range(gl):
                        nc.vector.bn_aggr(mv[:, i, :], stats[:, i, :])
                    rstd = pln.tile([128, GW], f32, tag="rstd",
                                    name="rstd")[:, :gl]
                    nc.scalar.activation(rstd, mv[:, :, 1], AF.Sqrt,
                                         bias=eps_sb[:, 0:1])
                    nc.vector.reciprocal(rstd, rstd)
                    nmr = pln.tile([128, GW], f32, tag="nmr",
                                   name="nmr")[:, :gl]
                    nc.vector.tensor_tensor(nmr, mv[:, :, 0], rstd,
                                            op=mybir.AluOpType.mult)
                    nc.vector.tensor_tensor(nmr, zero_sb[:, :gl], nmr,
                                            op=mybir.AluOpType.subtract)

                    # apply LN1 (+ReLU) on the scalar engine: relu(z*r - mu*r)
                    h = pcc.tile([128, GW * 128], bf, tag="h",
                                 name="h")[:, :gl * 128]
                    fn1 = AF.Identity if aff1 else AF.Relu
                    for i in range(gl):
                        nc.scalar.activation(
                            h[:, i * 128:(i + 1) * 128],
                            z1v[:, i * 128:(i + 1) * 128], fn1,
                            scale=rstd[:, i:i + 1], bias=nmr[:, i:i + 1])
                    if aff1:
                        h3 = h.rearrange("p (w f) -> p w f", f=128)
                        nc.vector.tensor_tensor(
                            h3, h3,
                            aff_sb["G1"][:, None, :].to_broadcast([128, gl, 128]),
                            op=mybir.AluOpType.mult)
                        nc.vector.tensor_tensor(
                            h3, h3,
                            aff_sb["B1L"][:, None, :].to_broadcast([128, gl, 128]),
                            op=mybir.AluOpType.add)
                        nc.scalar.activation(h, h, AF.Relu)

                    # transpose h, apply W2, residual
                    hT = pcc.tile([128, GW * 128], bf, tag="hT",
                                  name="hT")[:, :gl * 128]
                    for i in range(gl):
                        pst = psB.tile([128, 128], bf, tag="pst", name="pst")
                        nc.tensor.transpose(
                            pst, h[:, i * 128:(i + 1) * 128], id_sb)
                        nc.scalar.copy(hT[:, i * 128:(i + 1) * 128], pst)
                    xg = pcc.tile([128, GW * 128], f32, tag="xg",
                                  name="xg")[:, :gl * 128]
                    nc.sync.dma_start(
                        xg.rearrange("p (w f) -> p w f", f=128),
                        x_r[:, w0:w0 + gl, :])
                    og = pcc.tile([128, GW * 128], f32, tag="og",
                                  name="og")[:, :gl * 128]
                    for i in range(gl):
                        ps5 = psB.tile([128, 128], f32, tag="ps5", name="ps5")
                        nc.tensor.matmul(ps5,
                                         lhsT=hT[:, i * 128:(i + 1) * 128],
                                         rhs=W2_sb, start=True, stop=True)
                        nc.vector.tensor_add(og[:, i * 128:(i + 1) * 128],
                                             ps5, xg[:, i * 128:(i + 1) * 128])
                    og_3d = og.rearrange("p (w f) -> p w f", f=128)
                    if use_b2:
                        nc.vector.tensor_tensor(
                            og_3d, og_3d,
                            aff_sb["B2"][:, None, :].to_broadcast([128, gl, 128]),
                            op=mybir.AluOpType.add)

                    # LayerNorm 2
                    stats2 = pln.tile([128, GW, 6], f32, tag="bnst",
                                      name="stats2")[:, :gl, :]
                    for i in range(gl):
                        nc.vector.bn_stats(stats2[:, i, :],
                                           og[:, i * 128:(i + 1) * 128])
                    mv2 = pln.tile([128, GW, 2], f32, tag="bnmv",
                                   name="mv2")[:, :gl, :]
                    for i in range(gl):
                        nc.vector.bn_aggr(mv2[:, i, :], stats2[:, i, :])
                    rstd2 = pln.tile([128, GW], f32, tag="rstd",
                                     name="rstd2")[:, :gl]
                    nc.scalar.activation(rstd2, mv2[:, :, 1], AF.Sqrt,
                                         bias=eps_sb[:, 0:1])
                    nc.vector.reciprocal(rstd2, rstd2)
                    nmr2 = pln.tile([128, GW], f32, tag="nmr",
                                    name="nmr2")[:, :gl]
                    nc.vector.tensor_tensor(nmr2, mv2[:, :, 0], rstd2,
                                            op=mybir.AluOpType.mult)
                    nc.vector.tensor_tensor(nmr2, zero_sb[:, :gl], nmr2,
                                            op=mybir.AluOpType.subtract)

                    outg = pcc.tile([128, GW * 128], f32, tag="outg",
                                    name="outg")[:, :gl * 128]
                    for i in range(gl):
                        nc.scalar.activation(
                            outg[:, i * 128:(i + 1) * 128],
                            og[:, i * 128:(i + 1) * 128], AF.Identity,
                            scale=rstd2[:, i:i + 1], bias=nmr2[:, i:i + 1])
                    outg_3d = outg.rearrange("p (w f) -> p w f", f=128)
                    if aff2:
                        nc.vector.tensor_tensor(
                            outg_3d, outg_3d,
                            aff_sb["GN"][:, None, :].to_broadcast([128, gl, 128]),
                            op=mybir.AluOpType.mult)
                        nc.vector.tensor_tensor(
                            outg_3d, outg_3d,
                            aff_sb["BN"][:, None, :].to_broadcast([128, gl, 128]),
                            op=mybir.AluOpType.add)

                    nc.sync.dma_start(o_r[:, w0:w0 + gl, :], outg_3d)

                # warm up the Q7 gather ucode (first dma_gather pays a ~6us
                # IRAM load): junk 128-idx gather nobody reads
                widx = pcst.tile([128, 8], i16)
                nc.vector.memset(widx, 0)
                wG = pgG.tile([128, 1, 128], f32, tag="warmG", name="wG")
                nc.gpsimd.dma_gather(
                    out_ap=wG, in_ap=x_nm_t.ap()[:, :], idxs_ap=widx,
                    num_idxs=128, num_idxs_reg=128, elem_size=128,
                    queue_num=3)

                # Warm-up schedule: lead with gathers for ready tables while
                # the AllGather chain (serialized on the collective queue)
                # drains; AG triggers and table expands are interleaved so the
                # Pool engine never parks long on a collective-queue slot.
                open_group(0)
                open_group(1)
                open_group(2)
                emit_expand(0)
                emit_batches(0, (0,))
                emit_batches(1, (0,))
                emit_batches(2, (0,))
                emit_ag(2)
                emit_expand(1)
                emit_batches(0, (1,))
                emit_batches(1, (1,))
                emit_batches(2, (1,))
                emit_ag(3)
                emit_expand(2)
                for gi in (0, 1, 2):
                    emit_batches(gi, (2,))
                emit_expand(3)
                for gi in (0, 1, 2):
                    emit_batches(gi, (3,))
                    emit_phase_c(gi)
                for gi in range(3, NG):
                    open_group(gi)
                    emit_batches(gi, (0, 1, 2, 3))
                    emit_phase_c(gi)

    nc.compile()
    return nc


# ----------------------------------------------------------------------------
# entry points
# ----------------------------------------------------------------------------

def _assemble(results_list, perm, N, D):
    out = np.empty((N, D), np.float32)
    pc = perm.reshape(NCORES, -1)
    for c in range(NCORES):
        m = pc[c] >= 0
        out[pc[c][m]] = results_list[c][m]
    return out


def _install_ntff_hook_shim():
    """This image's antenv lacks axon_hooks; synthesize it so trace=True can
    reach the libaxon NTFF profiler (see trn_agent_boot.trn_boot)."""
    import types
    if "antenv.axon_hooks" in sys.modules:
        return
    try:
        from trn_agent_boot.trn_boot import _ntff_profile_via_ctypes
        hook = _ntff_profile_via_ctypes("/opt/axon/libaxon_pjrt.so")
    except Exception:
        hook = None
    mod = types.ModuleType("antenv.axon_hooks")
    state = {"hook": hook}
    mod.get_axon_ntff_profile_hook = lambda: state["hook"]
    mod.set_axon_ntff_profile_hook = lambda h: state.update(hook=h)
    sys.modules["antenv.axon_hooks"] = mod


def _run_hw(nc, in_maps, trace=False):
    if trace:
        sys.path.insert(0, "/root/.axon_site")
        _install_ntff_hook_shim()
    from concourse.bass_utils import run_bass_kernel_spmd
    res = run_bass_kernel_spmd(nc, in_maps, core_ids=list(range(NCORES)),
                               trace=trace)
    return res


def _run_sim(nc, in_maps):
    from concourse.bass_interp import MultiCoreSim
    sim = MultiCoreSim(nc, num_cores=NCORES, trace=False,
                       require_finite=False, require_nnan=False)
    cores = list(sim.cores.values())
    for c, core in enumerate(cores):
        for k, v in in_maps[c].items():
            core.tensor(k)[:] = v
    sim.simulate(check_with_hw=False)
    return [np.array(core.tensor("out")) for core in cores]


def kernel(**inputs) -> np.ndarray:
    cfg, weights, in_maps, perm, N = _prepare(inputs)
    nc = _build_nc(cfg, weights)
    res = _run_hw(nc, in_maps)
    outs = [res.results[c]["out"] for c in range(NCORES)]
    return _assemble(outs, perm, N, cfg["D"])


# revision 22
# speedup vs baseline: 1.0408x; 1.0022x over previous
"""Trainium2 Bass kernel for MinimalCopresheafTNN (GNN message passing).

Strategy (8 NeuronCores, SPMD single program):
  * Host: fold W_r / R[p] / W1 into one per-polarity matrix D_p = W_r.T @ R_p @ W1.T
    (linearity of segment_sum), fold res_scale into W2. Since b1 == 0, the
    per-node deg_norm scale cancels through LayerNorm1 (LN is invariant to
    positive per-row scaling), so it is dropped entirely. Permute nodes so
    each core owns a contiguous, polarity-grouped slice (segments padded to
    128 and uniform across cores).
  * Device, per core:
      Phase A: x_send = x @ S[pol] for the core's slice (bf16 matmuls),
               AllGather the full x_send table into HBM on every core.
      Phase B: SpMM agg = scatter-add of x_send[row] into the core's dest
               nodes: dma_gather of source rows (4 table quadrants, int16
               indices, max 1024/call) + one-hot matmul scatter into PSUM
               per 128-dest window.
      Phase C: z1 = aggT @ D_p -> LayerNorm(+ReLU on scalar engine) ->
               transpose -> @ (res*W2.T) + x -> LayerNorm -> out.
  * Host: inverse-permute per-core outputs into the full [N, D] result.
"""

import sys

import numpy as np

sys.path.insert(0, "/opt/trn_rl_repo")

NCORES = 8
LN_EPS = 1e-5
DMA_SCRATCH = 65536
GW = 4                           # windows per group (PSUM bank = 512 f32)
SUBB = 8                         # chunks per gather batch (1024 idx HW limit)
GBUFS = 12                       # G-tile pipeline depth


# ----------------------------------------------------------------------------
# host-side preparation
# ----------------------------------------------------------------------------

def _prepare(inputs):
    x = np.asarray(inputs["x"], np.float32)
    N, D = x.shape
    S = (np.asarray(inputs["send_maps"], np.float32)
         + np.asarray(inputs["delta_send"], np.float32))
    Rm = (np.asarray(inputs["receive_maps"], np.float32)
          + np.asarray(inputs["delta_receive"], np.float32))
    P = S.shape[0]
    W_r = np.asarray(inputs["W_r"], np.float32)
    W1 = np.asarray(inputs["W1"], np.float32)
    b1 = np.asarray(inputs["b1"], np.float32)
    ln1_g = np.asarray(inputs["ln1_g"], np.float32)
    ln1_b = np.asarray(inputs["ln1_b"], np.float32)
    W2 = np.asarray(inputs["W2"], np.float32)
    b2 = np.asarray(inputs["b2"], np.float32)
    norm_g = np.asarray(inputs["norm_g"], np.float32)
    norm_b = np.asarray(inputs["norm_b"], np.float32)
    res = float(np.asarray(inputs["res_scale"]))
    row = np.asarray(inputs["row"]).astype(np.int64)
    col = np.asarray(inputs["col"]).astype(np.int64)
    pols = np.asarray(inputs["ring_polarities"]).astype(np.int64) % P
    E = row.shape[0]

    deg = np.bincount(row, minlength=N).astype(np.float32)
    dn = (1.0 / np.maximum(deg, 1.0)).astype(np.float32)
    indeg = np.bincount(col, minlength=N)

    # deg_norm is a positive per-row scale applied before `@ W1 + b1` and
    # LayerNorm; when b1 == 0 LayerNorm's scale invariance cancels it.
    use_dn = not bool(np.all(b1 == 0))
    trivial_aff1 = bool(np.all(ln1_g == 1) and np.all(ln1_b == 0))
    trivial_aff2 = bool(np.all(norm_g == 1) and np.all(norm_b == 0))
    trivial_b2 = bool(np.all(b2 == 0))

    # --- node -> (core, position) assignment --------------------------------
    L = np.zeros(P, np.int64)              # padded segment length per polarity
    core_nodes = [[None] * P for _ in range(NCORES)]
    for p in range(P):
        nodes_p = np.where(pols == p)[0]
        order = nodes_p[np.argsort(-indeg[nodes_p], kind="stable")]
        mx = 0
        for c in range(NCORES):
            core_nodes[c][p] = order[c::NCORES]
            mx = max(mx, len(core_nodes[c][p]))
        L[p] = max(128, ((mx + 127) // 128) * 128)
    M = int(L.sum())
    M = ((M + 511) // 512) * 512          # quadrants must be block-aligned
    W = M // 128
    NP = NCORES * M
    MQ = M // 4
    Q = NCORES * MQ                       # rows per quadrant table
    assert Q <= 32767, f"quadrant rows {Q} exceed int16 range"

    seg_start = np.concatenate([[0], np.cumsum(L)[:-1]])
    pol_of_block = np.repeat(np.arange(P), L // 128)
    pol_of_block = np.concatenate(
        [pol_of_block, np.full(W - len(pol_of_block), P - 1, np.int64)])

    perm = np.full(NP, -1, dtype=np.int64)
    for c in range(NCORES):
        for p in range(P):
            nodes = core_nodes[c][p]
            n_w = L[p] // 128
            base = c * M + seg_start[p]
            j = np.arange(len(nodes))
            perm[base + (j % n_w) * 128 + j // n_w] = nodes
    real = perm >= 0
    pos_of = np.empty(N, dtype=np.int64)
    pos_of[perm[real]] = np.nonzero(real)[0]

    # --- edge layout --------------------------------------------------------
    col_pos = pos_of[col]
    row_pos = pos_of[row]
    core_e = col_pos // M
    w_e = (col_pos % M) // 128
    rel_e = (col_pos % M) % 128
    n_in_core = row_pos % M
    q_e = n_in_core // MQ
    rel_s = (row_pos // M) * MQ + (n_in_core % MQ)
    dn_e = dn[col]

    key = (core_e * W + w_e) * 4 + q_e
    cnt = np.bincount(key, minlength=NCORES * W * 4).reshape(NCORES, W, 4)
    C = np.maximum(1, -(-cnt.max(axis=0) // 128)).astype(np.int64)      # [W, 4]

    wgroups = [list(range(g, min(g + GW, W))) for g in range(0, W, GW)]

    chunk_start = np.zeros((W, 4), np.int64)
    chunk_w, chunk_q, chunk_k = [], [], []
    batches_by_group = []            # [gi] -> list of (q, ch0, ch1), <= SUBB
    group_ch0 = []                   # first chunk id of each group
    nch = 0
    for wg in wgroups:
        group_ch0.append(nch)
        gb = []
        for q in range(4):
            b0 = nch
            for w in wg:
                chunk_start[w, q] = nch
                for k in range(C[w, q]):
                    chunk_w.append(w)
                    chunk_q.append(q)
                    chunk_k.append(k)
                nch += C[w, q]
            for s0 in range(b0, nch, SUBB):
                gb.append((q, s0, min(s0 + SUBB, nch)))
        batches_by_group.append(gb)
    NCH = int(nch)
    EP = 128 * NCH
    group_nch = [(batches_by_group[gi][-1][2] - group_ch0[gi])
                 for gi in range(len(wgroups))]
    NCHG = max(group_nch)

    import ml_dtypes
    bf16 = ml_dtypes.bfloat16
    f8 = ml_dtypes.float8_e4m3
    idx_arr = np.zeros((NCORES, EP), np.int16)
    reld_arr = np.full((NCORES, 128, NCH), -1, np.int16)

    order_e = np.argsort(key, kind="stable")
    counts_flat = np.bincount(key, minlength=NCORES * W * 4)
    group_start = np.zeros(NCORES * W * 4 + 1, np.int64)
    group_start[1:] = np.cumsum(counts_flat)
    r = np.arange(E) - group_start[key[order_e]]
    c_of = core_e[order_e]
    tchunk = chunk_start[w_e[order_e], q_e[order_e]] + r // 128
    lane = r % 128
    s = tchunk * 128 + lane
    idx_arr[c_of, s] = rel_s[order_e].astype(np.int16)
    reld_arr[c_of, lane, tchunk] = rel_e[order_e].astype(np.int16)

    # Uniform trailing trim: the Q7 gather ucode skips trailing negative
    # indices, and num_idxs_reg must equal the non-negative count — trim
    # every batch at the max-over-cores last-real-edge position (identical on
    # all cores). First GBUFS batches untouched (first-use G slots may be NaN).
    occupied = np.zeros((NCORES, EP), bool)
    occupied[c_of, s] = True
    batch_cnt_by_group = []
    for gi, gb in enumerate(batches_by_group):
        cnts = []
        for (_, ch0, ch1) in gb:
            Lb = (ch1 - ch0) * 128
            # first-emitted groups left untrimmed (first-use G slots are NaN)
            if gi < 5:
                cnts.append(Lb)
                continue
            nz = np.nonzero(occupied[:, ch0 * 128:ch1 * 128].any(axis=0))[0]
            T = int(nz[-1] + 1) if len(nz) else 16
            T = min(Lb, ((T + 15) // 16) * 16)
            idx_arr[:, ch0 * 128 + T:ch1 * 128] = -1
            cnts.append(T)
        batch_cnt_by_group.append(cnts)

    # host-built one-hot scatter matrix (0/1 exact in fp8): [128, NCH, 128]
    hoh = (reld_arr[:, :, :, None]
           == np.arange(128, dtype=np.int16)[None, None, None, :]).astype(f8)

    # wrapped + replicated gather-index layout: idx i lives at [i%16, i//16],
    # replicated over the 8 Q7 partition groups
    idx_rep = np.empty((NCORES, 128, EP // 16), np.int16)
    for c in range(NCORES):
        idx_rep[c] = np.tile(idx_arr[c].reshape(EP // 16, 16).T, (8, 1))

    # --- per-core node data -------------------------------------------------
    x_nm = np.zeros((NCORES, M, D), np.float32)
    pc = perm.reshape(NCORES, M)
    for c in range(NCORES):
        m = pc[c] >= 0
        x_nm[c][m] = x[pc[c][m]]
    xT = np.ascontiguousarray(x_nm.transpose(0, 2, 1)).astype(bf16)

    # per-window deg_norm column (only used when b1 != 0)
    dn_nm = np.ones((NCORES, M), np.float32)
    for c in range(NCORES):
        m = pc[c] >= 0
        dn_nm[c][m] = dn[pc[c][m]]
    dn_cols = dn_nm.reshape(NCORES, W, 128).transpose(0, 2, 1).copy()

    # --- fused weights ------------------------------------------------------
    D_all = np.einsum(
        "de,pef,fg->pdg",
        W_r.T.astype(np.float64), Rm.astype(np.float64), W1.T.astype(np.float64),
    ).astype(np.float32)
    W2s = (res * W2.T).astype(np.float32)

    cfg = dict(
        D=D, P=P, M=M, W=W, NP=NP, Q=Q, MQ=MQ, NCH=NCH, EP=EP, NCHG=NCHG,
        pol_of_block=pol_of_block.tolist(),
        wgroups=wgroups, C=C, batches_by_group=batches_by_group,
        batch_cnt_by_group=batch_cnt_by_group, group_ch0=group_ch0,
        group_nch=group_nch,
        chunk_w=chunk_w, chunk_k=chunk_k,
        use_dn=use_dn, trivial_aff1=trivial_aff1, trivial_aff2=trivial_aff2,
        trivial_b2=trivial_b2,
    )
    weights = dict(
        S_all=np.ascontiguousarray(S.reshape(P * D, D)).astype(bf16),
        D_all=np.ascontiguousarray(D_all.reshape(P * D, D)).astype(bf16),
        W2s=np.ascontiguousarray(W2s).astype(bf16),
        IDENT=np.eye(128, dtype=np.float32).astype(bf16),
        B1ROW=np.tile(b1, (128, 1)).astype(np.float32),
        G1ROW=np.tile(ln1_g, (128, 1)).astype(np.float32),
        B1LROW=np.tile(ln1_b, (128, 1)).astype(np.float32),
        GNROW=np.tile(norm_g, (128, 1)).astype(np.float32),
        BNROW=np.tile(norm_b, (128, 1)).astype(np.float32),
        B2ROW=np.tile(res * b2, (128, 1)).astype(np.float32),
    )
    in_maps = [
        dict(x_nm=x_nm[c], xT=xT[c], idx=idx_rep[c], hoh=hoh[c],
             dncol=dn_cols[c])
        for c in range(NCORES)
    ]
    return cfg, weights, in_maps, perm, N


# ----------------------------------------------------------------------------
# device program
# ----------------------------------------------------------------------------

def _build_nc(cfg, weights):
    import concourse.bass as bass
    import concourse.mybir as mybir
    import concourse.tile as tile
    from concourse import bacc

    f32 = mybir.dt.float32
    bf = mybir.dt.bfloat16
    f8 = mybir.dt.float8e4
    i16 = mybir.dt.int16
    AF = mybir.ActivationFunctionType
    D, P, M, W = cfg["D"], cfg["P"], cfg["M"], cfg["W"]
    NP, Q, NCH, EP = cfg["NP"], cfg["Q"], cfg["NCH"], cfg["EP"]
    MQ, NCHG = cfg["MQ"], cfg["NCHG"]
    pol_of_block = cfg["pol_of_block"]
    wgroups, C = cfg["wgroups"], cfg["C"]
    batches_by_group = cfg["batches_by_group"]
    batch_cnt_by_group = cfg["batch_cnt_by_group"]
    group_ch0, group_nch = cfg["group_ch0"], cfg["group_nch"]
    chunk_w, chunk_k = cfg["chunk_w"], cfg["chunk_k"]
    use_dn = cfg["use_dn"]
    aff1, aff2 = not cfg["trivial_aff1"], not cfg["trivial_aff2"]
    use_b2 = not cfg["trivial_b2"]
    NG = len(wgroups)

    nc = bacc.Bacc("TRN2", target_bir_lowering=False, debug=False,
                   num_devices=NCORES, enable_asserts=False,
                   dynamic_dma_scratch_size=DMA_SCRATCH,
                   num_swdge_queues=4)

    x_nm_t = nc.dram_tensor("x_nm", [M, D], f32, kind="ExternalInput")
    xT_t = nc.dram_tensor("xT", [D, M], bf, kind="ExternalInput")
    idx_t = nc.dram_tensor("idx", [128, EP // 16], i16, kind="ExternalInput")
    hoh_t = nc.dram_tensor("hoh", [128, NCH, 128], f8, kind="ExternalInput")
    dn_t = nc.dram_tensor("dncol", [128, W], f32, kind="ExternalInput")
    out_t = nc.dram_tensor("out", [M, D], f32, kind="ExternalOutput")

    S_c = nc.inline_tensor(weights["S_all"], name="S_all")
    D_c = nc.inline_tensor(weights["D_all"], name="D_all")
    W2_c = nc.inline_tensor(weights["W2s"], name="W2s")
    ID_c = nc.inline_tensor(weights["IDENT"], name="IDENT")
    aff_c = {}
    if aff1:
        aff_c["G1"] = nc.inline_tensor(weights["G1ROW"], name="G1ROW")
        aff_c["B1L"] = nc.inline_tensor(weights["B1LROW"], name="B1LROW")
    if use_dn:
        aff_c["B1"] = nc.inline_tensor(weights["B1ROW"], name="B1ROW")
    if aff2:
        aff_c["GN"] = nc.inline_tensor(weights["GNROW"], name="GNROW")
        aff_c["BN"] = nc.inline_tensor(weights["BNROW"], name="BNROW")
    if use_b2:
        aff_c["B2"] = nc.inline_tensor(weights["B2ROW"], name="B2ROW")

    with tile.TileContext(nc) as tc:
        with tc.tile_pool(name="dram", bufs=1, space="DRAM") as dp, \
             tc.tile_pool(name="consts", bufs=1) as pcst:
            xsend_b = dp.tile([M, D], f8)
            tables_f8 = [dp.tile([Q, D], f8, addr_space="Shared",
                                 name=f"tablef8{q}") for q in range(4)]
            tables = [dp.tile([Q, D], bf, name=f"table{q}") for q in range(4)]

            S_sb = pcst.tile([128, P, 128], bf)
            nc.sync.dma_start(S_sb, S_c.ap().rearrange("(p d) e -> d p e", d=128))

            def emit_ag(q):
                nc.gpsimd.collective_compute(
                    "AllGather", mybir.AluOpType.bypass,
                    replica_groups=[list(range(NCORES))],
                    ins=[xsend_b[q * MQ:(q + 1) * MQ, :].opt()],
                    outs=[tables_f8[q].opt()])

            A_ROWS = Q // 128           # table rows per partition stripe
            EXS = 8                     # expansion sub-chunks per quadrant
            AS = A_ROWS // EXS

            def make_expand(pex):
                f8_r = [t[:, :].rearrange("(p a) c -> p a c", p=128)
                        for t in tables_f8]
                bf_r = [t[:, :].rearrange("(p a) c -> p a c", p=128)
                        for t in tables]

                def emit_expand(q):
                    # fp8 -> bf16 row expansion routed through SBUF: the
                    # AllGather moves 128B fp8 rows, the gather needs 256B
                    # bf16 rows. Contiguous per-partition stripes keep the
                    # DMA at 128 fat descriptors each way.
                    for s in range(EXS):
                        fin = pex.tile([128, AS, 128], f8, tag="exf",
                                       name="fin")
                        nc.sync.dma_start(
                            fin, f8_r[q][:, s * AS:(s + 1) * AS, :])
                        fout = pex.tile([128, AS, 128], bf, tag="exo",
                                        name="fout")
                        nc.vector.tensor_copy(fout, fin)
                        nc.sync.dma_start(
                            bf_r[q][:, s * AS:(s + 1) * AS, :], fout)
                return emit_expand

            # ---------------- Phase A: x_send + AllGather -------------------
            with tc.tile_pool(name="paX", bufs=1) as paX, \
                 tc.tile_pool(name="paE", bufs=4) as paE, \
                 tc.tile_pool(name="paP", bufs=4, space="PSUM") as paP:
                xT_sb = paX.tile([128, M], bf)
                for k in range(4):
                    nc.sync.dma_start(xT_sb[:, k * MQ:(k + 1) * MQ],
                                      xT_t.ap()[:, k * MQ:(k + 1) * MQ])
                blocks_per_q = W // 4
                for b in range(W):
                    ps = paP.tile([128, 128], f32, tag="xsps", name="ps")
                    nc.tensor.matmul(
                        ps, lhsT=xT_sb[:, b * 128:(b + 1) * 128],
                        rhs=S_sb[:, pol_of_block[b], :], start=True, stop=True)
                    ev = paE.tile([128, 128], f8, tag="xsev", name="ev")
                    nc.scalar.copy(ev, ps)
                    nc.sync.dma_start(xsend_b[b * 128:(b + 1) * 128, :], ev)
                    if (b + 1) % blocks_per_q == 0 and (b + 1) // blocks_per_q <= 2:
                        emit_ag((b + 1) // blocks_per_q - 1)

            # remaining constants (after Phase A so its DMAs queue first)
            idx_all = pcst.tile([128, EP // 16], i16)
            nc.sync.dma_start(idx_all, idx_t.ap())
            D_sb = pcst.tile([128, P, 128], bf)
            nc.sync.dma_start(D_sb, D_c.ap().rearrange("(p d) e -> d p e", d=128))
            W2_sb = pcst.tile([128, 128], bf)
            nc.sync.dma_start(W2_sb, W2_c.ap())
            id_sb = pcst.tile([128, 128], bf)
            nc.sync.dma_start(id_sb, ID_c.ap())
            eps_sb = pcst.tile([128, 1], f32)
            nc.vector.memset(eps_sb, LN_EPS)
            zero_sb = pcst.tile([128, GW], f32)
            nc.vector.memset(zero_sb, 0.0)
            dn_sb = None
            if use_dn:
                dn_sb = pcst.tile([128, W], f32)
                nc.sync.dma_start(dn_sb, dn_t.ap())
            aff_sb = {}
            for k, t in aff_c.items():
                aff_sb[k] = pcst.tile([128, 128], f32, name=f"aff_{k}")
                nc.sync.dma_start(aff_sb[k], t.ap())

            # ---------------- Phase B + C, fused per window-group -----------
            x_r = x_nm_t.ap().rearrange("(w p) f -> p w f", p=128)
            o_r = out_t.ap().rearrange("(w p) f -> p w f", p=128)

            with tc.tile_pool(name="pgG", bufs=GBUFS) as pgG, \
                 tc.tile_pool(name="pgH", bufs=3) as pgH, \
                 tc.tile_pool(name="pcc", bufs=4) as pcc, \
                 tc.tile_pool(name="pln", bufs=2) as pln, \
                 tc.tile_pool(name="pex", bufs=2) as pex, \
                 tc.tile_pool(name="psP", bufs=3, space="PSUM") as psP, \
                 tc.tile_pool(name="psZ", bufs=1, space="PSUM") as psZ, \
                 tc.tile_pool(name="psB", bufs=2, space="PSUM") as psB:
                emit_expand = make_expand(pex)
                state = {}
                ctr = dict(g=0)

                def open_group(gi):
                    gl = len(wgroups[gi])
                    nch_g = group_nch[gi]
                    pbig = psP.tile([128, GW * 128], f32, tag="spmm",
                                    name="pbig")[:, :gl * 128]
                    Hg = pgH.tile([128, NCHG, 128], f8, tag="H",
                                  name="Hg")[:, :nch_g, :]
                    nc.sync.dma_start(
                        Hg, hoh_t.ap()[:, group_ch0[gi]:group_ch0[gi] + nch_g, :])
                    state[gi] = (pbig, Hg)

                def emit_batches(gi, qs):
                    pbig, Hg = state[gi]
                    w0 = wgroups[gi][0]
                    gch0 = group_ch0[gi]
                    for bi, (q, ch0, ch1) in enumerate(batches_by_group[gi]):
                        if q not in qs:
                            continue
                        nch_b = ch1 - ch0
                        G = pgG.tile([128, SUBB, 128], bf, tag="G",
                                     name="G")[:, :nch_b, :]
                        nc.gpsimd.dma_gather(
                            out_ap=G, in_ap=tables[q][:, :],
                            idxs_ap=idx_all[:, ch0 * 8:ch1 * 8],
                            num_idxs=nch_b * 128,
                            num_idxs_reg=batch_cnt_by_group[gi][bi],
                            elem_size=128, queue_num=ctr["g"] % 4)
                        ctr["g"] += 1
                        for j in range(ch0, ch1):
                            w, k = chunk_w[j], chunk_k[j]
                            i = w - w0
                            # PSUM start=True clears the WHOLE bank — emit it
                            # only on the chronologically-first matmul into
                            # each bank; later slices then overwrite via
                            # cleared has_written bits and accumulate after.
                            nc.tensor.matmul(
                                pbig[:, i * 128:(i + 1) * 128],
                                lhsT=G[:, j - ch0, :], rhs=Hg[:, j - gch0, :],
                                start=(q == 0 and k == 0 and i == 0),
                                stop=(q == 3 and k == C[w][3] - 1),
                                skip_group_check=True)

                def emit_phase_c(gi):
                    pbig, _ = state.pop(gi)
                    wg = wgroups[gi]
                    gl = len(wg)
                    w0 = wg[0]
                    aggT = pcc.tile([128, GW * 128], bf, tag="aggT",
                                    name="aggT")[:, :gl * 128]
                    nc.scalar.copy(aggT, pbig)

                    # z1 = aggT.T @ D_p  ->  [dest(part), feat], one PSUM bank
                    psz = psZ.tile([128, GW * 128], f32, tag="psz",
                                   name="psz")[:, :gl * 128]
                    for i, w in enumerate(wg):
                        nc.tensor.matmul(
                            psz[:, i * 128:(i + 1) * 128],
                            lhsT=aggT[:, i * 128:(i + 1) * 128],
                            rhs=D_sb[:, pol_of_block[w], :],
                            start=(i == 0), stop=True, skip_group_check=True)

                    z1v = psz
                    if use_dn or aff1:
                        # fallback: materialize z1 in SBUF to apply dn/b1
                        z1f = pcc.tile([128, GW * 128], f32, tag="z1f",
                                       name="z1f")[:, :gl * 128]
                        for i, w in enumerate(wg):
                            sc = dn_sb[:, w:w + 1] if use_dn else 1.0
                            nc.scalar.activation(
                                z1f[:, i * 128:(i + 1) * 128],
                                psz[:, i * 128:(i + 1) * 128],
                                AF.Identity, scale=sc)
                        if use_dn:
                            z3 = z1f.rearrange("p (w f) -> p w f", f=128)
                            nc.vector.tensor_tensor(
                                z3, z3,
                                aff_sb["B1"][:, None, :]
                                .to_broadcast([128, gl, 128]),
                                op=mybir.AluOpType.add)
                        z1v = z1f

                    # LayerNorm 1 stats
                    stats = pln.tile([128, GW, 6], f32, tag="bnst",
                                     name="stats")[:, :gl, :]
                    for i in range(gl):
                        nc.vector.bn_stats(stats[:, i, :],
                                           z1v[:, i * 128:(i + 1) * 128])
                    mv = pln.tile([128, GW, 2], f32, tag="bnmv",
                                  name="mv")[:, :gl, :]
                    for i in range(gl):
                        nc.vector.bn_aggr(mv[:, i, :], stats[:, i, :])
                    rstd = pln.tile([128, GW], f32, tag="rstd",
                                    name="rstd")[:, :gl]
                    nc.scalar.activation(rstd, mv[:, :, 1], AF.Sqrt,
                                         bias=eps_sb[:, 0:1])
                    nc.vector.reciprocal(rstd, rstd)
                    nmr = pln.tile([128, GW], f32, tag="nmr",
                                   name="nmr")[:, :gl]
                    nc.vector.tensor_tensor(nmr, mv[:, :, 0], rstd,
                                            op=mybir.AluOpType.mult)
                    nc.vector.tensor_tensor(nmr, zero_sb[:, :gl], nmr,
                                            op=mybir.AluOpType.subtract)

                    # apply LN1 (+ReLU) on the scalar engine: relu(z*r - mu*r)
                    h = pcc.tile([128, GW * 128], bf, tag="h",
                                 name="h")[:, :gl * 128]
                    fn1 = AF.Identity if aff1 else AF.Relu
                    for i in range(gl):
                        nc.scalar.activation(
                            h[:, i * 128:(i + 1) * 128],
                            z1v[:, i * 128:(i + 1) * 128], fn1,
                            scale=rstd[:, i:i + 1], bias=nmr[:, i:i + 1])
                    if aff1:
                        h3 = h.rearrange("p (w f) -> p w f", f=128)
                        nc.vector.tensor_tensor(
                            h3, h3,
                            aff_sb["G1"][:, None, :].to_broadcast([128, gl, 128]),
                            op=mybir.AluOpType.mult)
                        nc.vector.tensor_tensor(
                            h3, h3,
                            aff_sb["B1L"][:, None, :].to_broadcast([128, gl, 128]),
                            op=mybir.AluOpType.add)
                        nc.scalar.activation(h, h, AF.Relu)

                    # transpose h, apply W2, residual
                    hT = pcc.tile([128, GW * 128], bf, tag="hT",
                                  name="hT")[:, :gl * 128]
                    for i in range(gl):
                        pst = psB.tile([128, 128], bf, tag="pst", name="pst")
                        nc.tensor.transpose(
                            pst, h[:, i * 128:(i + 1) * 128], id_sb)
                        nc.scalar.copy(hT[:, i * 128:(i + 1) * 128], pst)
                    xg = pcc.tile([128, GW * 128], f32, tag="xg",
                                  name="xg")[:, :gl * 128]
                    nc.sync.dma_start(
                        xg.rearrange("p (w f) -> p w f", f=128),
                        x_r[:, w0:w0 + gl, :])
                    og = pcc.tile([128, GW * 128], f32, tag="og",
                                  name="og")[:, :gl * 128]
                    for i in range(gl):
                        ps5 = psB.tile([128, 128], f32, tag="ps5", name="ps5")
                        nc.tensor.matmul(ps5,
                                         lhsT=hT[:, i * 128:(i + 1) * 128],
                                         rhs=W2_sb, start=True, stop=True)
                        nc.vector.tensor_add(og[:, i * 128:(i + 1) * 128],
                                             ps5, xg[:, i * 128:(i + 1) * 128])
                    og_3d = og.rearrange("p (w f) -> p w f", f=128)
                    if use_b2:
                        nc.vector.tensor_tensor(
                            og_3d, og_3d,
                            aff_sb["B2"][:, None, :].to_broadcast([128, gl, 128]),
                            op=mybir.AluOpType.add)

                    # LayerNorm 2
                    stats2 = pln.tile([128, GW, 6], f32, tag="bnst",
                                      name="stats2")[:, :gl, :]
                    for i in range(gl):
                        nc.vector.bn_stats(stats2[:, i, :],
                                           og[:, i * 128:(i + 1) * 128])
                    mv2 = pln.tile([128, GW, 2], f32, tag="bnmv",
                                   name="mv2")[:, :gl, :]
                    for i in range(gl):
                        nc.vector.bn_aggr(mv2[:, i, :], stats2[:, i, :])
                    rstd2 = pln.tile([128, GW], f32, tag="rstd",
                                     name="rstd2")[:, :gl]
                    nc.scalar.activation(rstd2, mv2[:, :, 1], AF.Sqrt,
                                         bias=eps_sb[:, 0:1])
                    nc.vector.reciprocal(rstd2, rstd2)
                    nmr2 = pln.tile([128, GW], f32, tag="nmr",
                                    name="nmr2")[:, :gl]
                    nc.vector.tensor_tensor(nmr2, mv2[:, :, 0], rstd2,
                                            op=mybir.AluOpType.mult)
                    nc.vector.tensor_tensor(nmr2, zero_sb[:, :gl], nmr2,
                                            op=mybir.AluOpType.subtract)

                    outg = pcc.tile([128, GW * 128], f32, tag="outg",
                                    name="outg")[:, :gl * 128]
                    for i in range(gl):
                        nc.scalar.activation(
                            outg[:, i * 128:(i + 1) * 128],
                            og[:, i * 128:(i + 1) * 128], AF.Identity,
                            scale=rstd2[:, i:i + 1], bias=nmr2[:, i:i + 1])
                    outg_3d = outg.rearrange("p (w f) -> p w f", f=128)
                    if aff2:
                        nc.vector.tensor_tensor(
                            outg_3d, outg_3d,
                            aff_sb["GN"][:, None, :].to_broadcast([128, gl, 128]),
                            op=mybir.AluOpType.mult)
                        nc.vector.tensor_tensor(
                            outg_3d, outg_3d,
                            aff_sb["BN"][:, None, :].to_broadcast([128, gl, 128]),
                            op=mybir.AluOpType.add)

                    nc.sync.dma_start(o_r[:, w0:w0 + gl, :], outg_3d)

                # warm up the Q7 gather ucode (first dma_gather pays a ~6us
                # IRAM load): junk 128-idx gather nobody reads
                widx = pcst.tile([128, 8], i16)
                nc.vector.memset(widx, 0)
                wG = pgG.tile([128, 1, 128], f32, tag="warmG", name="wG")
                nc.gpsimd.dma_gather(
                    out_ap=wG, in_ap=x_nm_t.ap()[:, :], idxs_ap=widx,
                    num_idxs=128, num_idxs_reg=128, elem_size=128,
                    queue_num=3)

                # Warm-up schedule: lead with gathers for ready tables while
                # the AllGather chain (serialized on the collective queue)
                # drains; AG triggers and table expands are interleaved so the
                # Pool engine never parks long on a collective-queue slot.
                open_group(0)
                open_group(1)
                open_group(2)
                emit_expand(0)
                emit_batches(0, (0,))
                emit_batches(1, (0,))
                emit_batches(2, (0,))
                emit_ag(2)
                emit_expand(1)
                emit_batches(0, (1,))
                emit_batches(1, (1,))
                emit_batches(2, (1,))
                emit_ag(3)
                emit_expand(2)
                for gi in (0, 1, 2):
                    emit_batches(gi, (2,))
                emit_expand(3)
                for gi in (0, 1, 2):
                    emit_batches(gi, (3,))
                    emit_phase_c(gi)
                for gi in range(3, NG):
                    open_group(gi)
                    emit_batches(gi, (0, 1, 2, 3))
                    emit_phase_c(gi)

    nc.compile()
    return nc


# ----------------------------------------------------------------------------
# entry points
# ----------------------------------------------------------------------------

def _assemble(results_list, perm, N, D):
    out = np.empty((N, D), np.float32)
    pc = perm.reshape(NCORES, -1)
    for c in range(NCORES):
        m = pc[c] >= 0
        out[pc[c][m]] = results_list[c][m]
    return out


def _install_ntff_hook_shim():
    """This image's antenv lacks axon_hooks; synthesize it so trace=True can
    reach the libaxon NTFF profiler (see trn_agent_boot.trn_boot)."""
    import types
    if "antenv.axon_hooks" in sys.modules:
        return
    try:
        from trn_agent_boot.trn_boot import _ntff_profile_via_ctypes
        hook = _ntff_profile_via_ctypes("/opt/axon/libaxon_pjrt.so")
    except Exception:
        hook = None
    mod = types.ModuleType("antenv.axon_hooks")
    state = {"hook": hook}
    mod.get_axon_ntff_profile_hook = lambda: state["hook"]
    mod.set_axon_ntff_profile_hook = lambda h: state.update(hook=h)
    sys.modules["antenv.axon_hooks"] = mod


def _run_hw(nc, in_maps, trace=False):
    if trace:
        sys.path.insert(0, "/root/.axon_site")
        _install_ntff_hook_shim()
    from concourse.bass_utils import run_bass_kernel_spmd
    res = run_bass_kernel_spmd(nc, in_maps, core_ids=list(range(NCORES)),
                               trace=trace)
    return res


def _run_sim(nc, in_maps):
    from concourse.bass_interp import MultiCoreSim
    sim = MultiCoreSim(nc, num_cores=NCORES, trace=False,
                       require_finite=False, require_nnan=False)
    cores = list(sim.cores.values())
    for c, core in enumerate(cores):
        for k, v in in_maps[c].items():
            core.tensor(k)[:] = v
    sim.simulate(check_with_hw=False)
    return [np.array(core.tensor("out")) for core in cores]


def kernel(**inputs) -> np.ndarray:
    cfg, weights, in_maps, perm, N = _prepare(inputs)
    nc = _build_nc(cfg, weights)
    res = _run_hw(nc, in_maps)
    outs = [res.results[c]["out"] for c in range(NCORES)]
    return _assemble(outs, perm, N, cfg["D"])
